# revision 6
# baseline (speedup 1.0000x reference)
"""NeRF render kernel v2 for 8 TRN2 NeuronCores (pure data parallel over rays).

Key speedups over v1 (810us -> ~544us timeline-sim):
- Hidden layers + heads in fp8-e4m3 (natural scale, no quant scaling needed);
  hidden layers use DoubleRow matmuls (full K=256 contraction in one MM).
  Sample 63 (1e10-delta, ReLU-sign-critical) stays full fp32: sample 62 runs
  as a solo fp8 stream and 63's fp32 layers are spread one-per-group across
  the schedule to avoid a serialization burst.
- Turns-domain Fourier encoding: the enc matmul computes u = pos*2^k/(2pi)
  (+0.25 for cos via a constant-1 moving row); range reduction is one ACT
  Copy (+MAGIC round) plus one DVE scalar_tensor_tensor giving round(u)-u,
  and Sin(scale=-2pi) restores the sign: sin(2pi*u) exactly (1-periodic).
- Encodings packed 2 samples per [124,512] tile; enc/L0 matmuls packed on
  row strips 0/32/64/96 (concurrent on HW via per-subarray row groups).
- ReLU+bias+fp8-quantize is one op per layer half, alternating DVE/ACT.
- Ray-position math, bulk weight-load DMA triggers, and SBUF-only composite
  ops run on the otherwise-idle GPSIMD engine; depth deltas precomputed.
- Head outputs batched 4 samples per PSUM bank (col strips), one copy + 4
  strided scatter DMAs per group. Exp/Tanh table set pre-warmed.
"""
import os
import numpy as np
import ml_dtypes

NB = 10
ENC = 60
WIDTH = 256
S = 64
RPC = 512
N_CORES = 8
NEAR, FAR = 0.1, 4.0
TWO_PI = float(2 * np.pi)
MAGIC = float(1.5 * 2 ** 23)

LAST_EXEC_NS = None
_CACHE = {}


def _build_nc():
    import concourse.bacc as bacc
    import concourse.tile as tile
    from concourse import mybir

    dt = mybir.dt
    AF = mybir.ActivationFunctionType
    ALU = mybir.AluOpType
    f32 = dt.float32
    f32r = dt.float32r
    fp8 = dt.float8e4
    DR = mybir.MatmulPerfMode.DoubleRow

    nc = bacc.Bacc("TRN2", target_bir_lowering=False, debug=False,
                   num_devices=N_CORES)

    def din(name, shape, dtype=f32):
        return nc.dram_tensor(name, shape, dtype, kind="ExternalInput")

    d_jit = din("jitter_t", [S, RPC])
    d_rp4 = din("rp4", [4, RPC])        # rows: rp0,rp1,rp2,ones
    d_rd4 = din("rd4", [4, RPC])        # rows: rd0,rd1,rd2,zeros
    d_brep = din("brep", [128, ENC])    # 2^k/2pi pattern + 0.25 cos row
    d_win_r = din("win_r", [128, WIDTH], f32r)  # negated perm'd w_in, rows 0-59 & 64-123
    d_win_32 = din("win_32", [128, WIDTH])      # same data, fp32 for the 62/63 pair
    d_whid8 = din("whid8", [128, 2, 14 * 128], fp8)
    d_whd8 = din("whd8", [128, 2, 16], fp8)
    d_whid32 = din("whid32", [128, 7 * 2 * WIDTH])
    d_whd32 = din("whd32", [128, 8])
    d_ball = din("ball", [128, 16])
    d_bca = din("bca", [128, 1])
    d_bcb = din("bcb", [S, 1])
    d_bcd = din("bcd", [S, 1])
    d_iota = din("iota", [S, 1])
    d_tris = din("tris", [S, S])
    d_onesb = din("onesb", [128, 2])
    d_big = din("big", [1, RPC])
    d_out = nc.dram_tensor("out", [4, RPC], f32, kind="ExternalOutput")

    DEBUG = bool(os.environ.get("KERNEL_DEBUG"))
    dbg = {}
    if DEBUG:
        for nm, shp, dt_ in (("dbg_jpos", [128, RPC], f32), ("dbg_ue", [128, RPC], f32),
                             ("dbg_fr", [124, RPC], f32), ("dbg_enc", [124, RPC], f32),
                             ("dbg_x0", [128, 2, RPC], fp8), ("dbg_x7", [128, 2, RPC], fp8),
                             ("dbg_stg", [100, RPC], f32), ("dbg_den", [S, RPC], f32),
                             ("dbg_tau", [S, RPC], f32), ("dbg_wt", [S, RPC], f32)):
            dbg[nm] = nc.dram_tensor(nm, shp, dt_, kind="ExternalOutput")

    with tile.TileContext(nc) as tc:
        with (
            tc.tile_pool(name="static", bufs=1) as sp,
            tc.tile_pool(name="act", bufs=6) as ap,
            tc.tile_pool(name="jpos", bufs=16) as jp,
            tc.tile_pool(name="comp", bufs=1) as cp,
            tc.tile_pool(name="ps_m", bufs=1, space="PSUM") as pm_,
            tc.tile_pool(name="ps_h", bufs=1, space="PSUM") as ph_,
            tc.tile_pool(name="ps_l", bufs=6, space="PSUM") as pl,
        ):
            def load(dram, shape, dtype, tag):
                t = sp.tile(shape, dtype, tag=tag)
                nc.sync.dma_start(t[:], dram[:])
                return t

            # startup-critical loads first (gate the first encodings / L0)
            jt = load(d_jit, [S, RPC], f32, "jt")
            iota = load(d_iota, [S, 1], f32, "iota")
            brep = load(d_brep, [128, ENC], f32, "brep")
            win_r = load(d_win_r, [128, WIDTH], f32r, "win_r")
            win_32 = load(d_win_32, [128, WIDTH], f32, "win_32")
            ball = load(d_ball, [128, 16], f32, "ball")
            rp128 = sp.tile([128, RPC], f32, tag="rp128")
            rd128 = sp.tile([128, RPC], f32, tag="rd128")
            for j in range(4):
                nc.sync.dma_start(rp128[32 * j:32 * j + 4, :], d_rp4[:, :])
                nc.sync.dma_start(rd128[32 * j:32 * j + 4, :], d_rd4[:, :])

            # depths = 0.1 + (3.9 * (idx + jitter)) / 64  (exact fp32 op order)
            # on DVE: it is idle at startup and SBUF-only ts ops run at 2x
            ddtmp = sp.tile([S, RPC], f32, tag="ddtmp")
            nc.vector.tensor_scalar(ddtmp[:], jt[:], iota[:], 3.9, ALU.add, ALU.mult)
            dd = sp.tile([S, RPC], f32, tag="dd")
            nc.vector.tensor_scalar(dd[:], ddtmp[:], float(1.0 / 64), 0.1, ALU.mult, ALU.add)

            def load_g(dram, shape, dtype, tag):
                t = sp.tile(shape, dtype, tag=tag)
                nc.gpsimd.dma_start(t[:], dram[:])
                return t

            whid8 = load_g(d_whid8, [128, 2, 14 * 128], fp8, "whid8")
            whd8 = load_g(d_whd8, [128, 2, 16], fp8, "whd8")
            whid32 = load_g(d_whid32, [128, 7 * 2 * WIDTH], f32, "whid32")
            whd32 = load_g(d_whd32, [128, 8], f32, "whd32")
            bca = load_g(d_bca, [128, 1], f32, "bca")
            bcb = load_g(d_bcb, [S, 1], f32, "bcb")
            bcd = load_g(d_bcd, [S, 1], f32, "bcd")
            tris = load_g(d_tris, [S, S], f32, "tris")
            onesb = load_g(d_onesb, [128, 2], f32, "onesb")

            # deltas depend only on depths: compute them up front
            ddsh = sp.tile([S, RPC], f32, tag="ddsh")
            nc.sync.dma_start(ddsh[0:63, :], dd[1:64, :])
            nc.sync.dma_start(ddsh[63:64, :], d_big[:])
            delt = sp.tile([S, RPC], f32, tag="delt")
            nc.vector.tensor_tensor(delt[:], ddsh[:], dd[:], ALU.subtract)

            # composite accumulators
            rgba = cp.tile([128, RPC], f32, tag="rgba")   # rows 0-63 rgb0, 64-127 rgb1
            rgbb = cp.tile([128, RPC], f32, tag="rgbb")   # rows 0-63 rgb2, 64-127 depths
            den = cp.tile([S, RPC], f32, tag="den")

            eng_ctr = [0]

            def relu_half(dst, src, bias_col, use_dve=None):
                """dst = fp8/f32(relu(src + bias)); alternates DVE/ACT."""
                if use_dve is None:
                    use_dve = bool(eng_ctr[0] & 1)
                    eng_ctr[0] += 1
                if use_dve:
                    nc.vector.tensor_scalar(dst, src, bias_col, 0.0, ALU.add, ALU.max)
                else:
                    nc.scalar.activation(dst, src, AF.Relu, bias=bias_col)

            def emit_pair(sA, sB, enc, g):
                """MLP for a 2-sample fp8 pair (f32r L0, DoubleRow hidden)."""
                xA = ap.tile([128, 2, RPC], fp8, tag="x8a")
                xB = ap.tile([128, 2, RPC], fp8, tag="x8b")
                for l in range(8):
                    pms = []
                    for mc in range(2):
                        pA = pl.tile([128, RPC], f32, tag="lp", name="pA")
                        pB = pl.tile([128, RPC], f32, tag="lp", name="pB")
                        if l == 0:
                            nc.tensor.matmul(pA[:], win_r[0:ENC, mc * 128:(mc + 1) * 128],
                                             enc[0:ENC, :], start=True, stop=True,
                                             tile_position=(0, 0))
                            nc.tensor.matmul(pB[:], win_r[64:64 + ENC, mc * 128:(mc + 1) * 128],
                                             enc[64:64 + ENC, :], start=True, stop=True,
                                             tile_position=(64, 0))
                        else:
                            ch = ((l - 1) * 2 + mc) * 128
                            nc.tensor.matmul(pA[:], whid8[:, :, ch:ch + 128],
                                             xA[:, :, :], start=True, stop=True,
                                             perf_mode=DR)
                            nc.tensor.matmul(pB[:], whid8[:, :, ch:ch + 128],
                                             xB[:, :, :], start=True, stop=True,
                                             perf_mode=DR)
                        pms.append((pA, pB))
                    nxA = ap.tile([128, 2, RPC], fp8, tag="x8a")
                    nxB = ap.tile([128, 2, RPC], fp8, tag="x8b")
                    for mc in range(2):
                        pA, pB = pms[mc]
                        col = ball[:, 2 * l + mc:2 * l + mc + 1]
                        relu_half(nxA[:, mc, :], pA[:], col)
                        relu_half(nxB[:, mc, :], pB[:], col)
                    xA, xB = nxA, nxB
                    if DEBUG and sA == 0 and l == 0:
                        nc.sync.dma_start(dbg["dbg_x0"][:], xA[:])
                    if DEBUG and sA == 0 and l == 7:
                        nc.sync.dma_start(dbg["dbg_x7"][:], xA[:])
                hpg = _head_ps[g]
                qA, qB = 32 * (sA % 4), 32 * (sB % 4)
                for kc in range(2):
                    nc.tensor.matmul(hpg[qA:qA + 4, :], whd8[:, kc, 0:4], xA[:, kc, :],
                                     start=(kc == 0), stop=(kc == 1),
                                     tile_position=(0, qA))
                for kc in range(2):
                    nc.tensor.matmul(hpg[qB:qB + 4, :], whd8[:, kc, 0:4], xB[:, kc, :],
                                     start=(kc == 0), stop=(kc == 1),
                                     tile_position=(0, qB))

            def emit_62_solo(enc63, g):
                """Sample 62 alone: fp8 MLP off the shared f32 enc63 tile."""
                xA = ap.tile([128, 2, RPC], fp8, tag="x8a")
                for l in range(8):
                    pms = []
                    for mc in range(2):
                        pA = pl.tile([128, RPC], f32, tag="lp", name="pA")
                        if l == 0:
                            nc.tensor.matmul(pA[:], win_32[0:ENC, mc * 128:(mc + 1) * 128],
                                             enc63[0:ENC, :], start=True, stop=True,
                                             tile_position=(0, 0))
                        else:
                            ch = ((l - 1) * 2 + mc) * 128
                            nc.tensor.matmul(pA[:], whid8[:, :, ch:ch + 128],
                                             xA[:, :, :], start=True, stop=True,
                                             perf_mode=DR)
                        pms.append(pA)
                    nxA = ap.tile([128, 2, RPC], fp8, tag="x8a")
                    for mc in range(2):
                        col = ball[:, 2 * l + mc:2 * l + mc + 1]
                        relu_half(nxA[:, mc, :], pms[mc][:], col, mc == 0)
                    xA = nxA
                hpg = _head_ps[g]
                for kc in range(2):
                    nc.tensor.matmul(hpg[64:68, :], whd8[:, kc, 0:4], xA[:, kc, :],
                                     start=(kc == 0), stop=(kc == 1),
                                     tile_position=(0, 64))

            x63 = [None]

            def emit_63_layer(l, enc63):
                """One fp32 layer of sample 63, interleaved into the schedule."""
                nxt = sp.tile([128, 2, RPC], f32, tag=f"x63_{l % 2}", name="x63")
                pms = []
                for mc in range(2):
                    pB = pl.tile([128, RPC], f32, tag="lp", name="p63")
                    if l == 0:
                        nc.tensor.matmul(pB[:], win_32[64:64 + ENC, mc * 128:(mc + 1) * 128],
                                         enc63[64:64 + ENC, :], start=True, stop=True,
                                         tile_position=(64, 0))
                    else:
                        for kc in range(2):
                            col = ((l - 1) * 2 + kc) * WIDTH + mc * 128
                            nc.tensor.matmul(pB[:], whid32[:, col:col + 128],
                                             x63[0][:, kc, :], start=(kc == 0), stop=(kc == 1))
                    pms.append(pB)
                for mc in range(2):
                    col = ball[:, 2 * l + mc:2 * l + mc + 1]
                    relu_half(nxt[:, mc, :], pms[mc][:], col, mc == 0)
                x63[0] = nxt

            def emit_63_heads():
                hp63 = pl.tile([128, RPC], f32, tag="lp", name="hp63")
                for kc in range(2):
                    nc.tensor.matmul(hp63[0:4, :], whd32[:, kc * 4:kc * 4 + 4],
                                     x63[0][:, kc, :], start=(kc == 0), stop=(kc == 1),
                                     tile_position=(0, 0))
                s63 = cp.tile([4, RPC], f32, tag="stg63")
                nc.scalar.copy(s63[:], hp63[0:4, :])
                nc.sync.dma_start(rgba[63:64, :], s63[0:1, :])
                nc.sync.dma_start(rgba[127:128, :], s63[1:2, :])
                nc.sync.dma_start(rgbb[63:64, :], s63[2:3, :])
                nc.sync.dma_start(den[63:64, :], s63[3:4, :])

            _head_ps = {}
            gorder = [15] + list(range(15))
            jposs = {}
            for idx, g in enumerate(gorder):
                s0 = 4 * g
                dd4 = ap.tile([128, RPC], f32, tag="dd4")
                # i=3 rows multiply rd128 rows that are 0; any finite fill works
                # (they must be written: x*0 of uninitialized NaN poisons jpos)
                for i in range(4):
                    nc.sync.dma_start(dd4[i::32, :], dd[s0:s0 + 4, :])
                eng = nc.vector if idx < 2 else nc.gpsimd
                jtmp = ap.tile([128, RPC], f32, tag="jtmp")
                eng.tensor_tensor(jtmp[:], dd4[:], rd128[:], ALU.mult)
                jpos = jp.tile([128, RPC], f32, tag="jpos", name="jpos")
                eng.tensor_tensor(jpos[:], jtmp[:], rp128[:], ALU.add)
                if DEBUG and g == 0:
                    nc.sync.dma_start(dbg["dbg_jpos"][:], jpos[:])
                jposs[g] = jpos

            enc63 = sp.tile([124, RPC], f32, tag="enc63")
            for gi, g in enumerate(gorder):
                if 1 <= gi <= 8:
                    emit_63_layer(gi - 1, enc63)
                elif gi == 9:
                    emit_63_heads()
                s0 = 4 * g
                jpos = jposs[g]
                _head_ps[g] = ph_.tile([128, RPC], f32, tag="hp", name="hpg")

                ues = []
                for pr in range(2):
                    jA, jB = 2 * pr, 2 * pr + 1
                    ue = pm_.tile([128, RPC], f32, tag="m", name="ue")
                    nc.tensor.matmul(ue[0:ENC, :], brep[32 * jA:32 * jA + 4, :],
                                     jpos[32 * jA:32 * jA + 4, :], start=True,
                                     stop=True, tile_position=(32 * jA, 0))
                    nc.tensor.matmul(ue[64:64 + ENC, :], brep[32 * jB:32 * jB + 4, :],
                                     jpos[32 * jB:32 * jB + 4, :], start=True,
                                     stop=True, tile_position=(32 * jB, 64))
                    ues.append(ue)
                for pr in range(2):
                    sA, sB = s0 + 2 * pr, s0 + 2 * pr + 1
                    # rnm = fl(u + MAGIC) on ACT (Copy is exact); then on DVE
                    # fr_neg = (rnm - MAGIC) - u = round(u) - u; Sin scale -2pi
                    # flips the sign back: sin(2pi*(u - round(u))) = sin(2pi*u).
                    rnm = ap.tile([124, RPC], f32, tag="rnm")
                    nc.scalar.activation(rnm[:], ues[pr][0:124, :], AF.Copy, bias=MAGIC)
                    fr = ap.tile([124, RPC], f32, tag="fr")
                    nc.vector.scalar_tensor_tensor(fr[:], rnm[:], -MAGIC, ues[pr][0:124, :],
                                                   ALU.add, ALU.subtract)
                    enc = enc63 if sB == 63 else ap.tile([124, RPC], f32r, tag="enc")
                    nc.scalar.activation(enc[:], fr[:], AF.Sin, scale=-TWO_PI)
                    if DEBUG and sA == 0:
                        uec = ap.tile([128, RPC], f32, tag="uec")
                        nc.vector.tensor_copy(uec[:], ues[pr][:])
                        nc.sync.dma_start(dbg["dbg_ue"][:], uec[:])
                        nc.sync.dma_start(dbg["dbg_fr"][:], fr[:])
                        nc.sync.dma_start(dbg["dbg_enc"][:], enc[:].bitcast(f32))
                    if sB == 63:
                        emit_62_solo(enc63, g)
                    else:
                        emit_pair(sA, sB, enc, g)

                # stg copy + scatter for the whole group
                stg = ap.tile([100, RPC], f32, tag="stg")
                nc.scalar.copy(stg[:], _head_ps[g][0:100, :])
                if DEBUG and g == 0:
                    nc.sync.dma_start(dbg["dbg_stg"][:], stg[:])
                nc.sync.dma_start(rgba[s0:s0 + 4, :], stg[0::32, :])
                nc.sync.dma_start(rgba[S + s0:S + s0 + 4, :], stg[1::32, :])
                nc.sync.dma_start(rgbb[s0:s0 + 4, :], stg[2::32, :])
                nc.sync.dma_start(den[s0:s0 + 4, :], stg[3::32, :])

            # warm the exp/tanh activation table set off the critical tail
            warm = cp.tile([1, 8], f32, tag="warm")
            nc.scalar.activation(warm[:], dd[0:1, 0:8], AF.Exp)
            nc.scalar.activation(warm[:], dd[0:1, 0:8], AF.Tanh)

            # ---- head activations ----
            # rgb = 0.5 + 0.5*tanh(0.5*z + 0.5*b_rgb); den = relu(z + b_den)
            tmpa = cp.tile([128, RPC], f32, tag="tmpa")
            nc.scalar.activation(tmpa[:], rgba[:], AF.Tanh, bias=bca[:], scale=0.5)
            nc.gpsimd.tensor_scalar(rgba[:], tmpa[:], 0.5, 0.5, ALU.mult, ALU.add)
            tmpb = cp.tile([S, RPC], f32, tag="tmpb")
            nc.scalar.activation(tmpb[:], rgbb[0:S, :], AF.Tanh, bias=bcb[:], scale=0.5)
            nc.gpsimd.tensor_scalar(rgbb[0:S, :], tmpb[:], 0.5, 0.5, ALU.mult, ALU.add)
            denr = cp.tile([S, RPC], f32, tag="denr")
            nc.vector.tensor_scalar(denr[:], den[:], bcd[:], 0.0, ALU.add, ALU.max)

            # ---- volume rendering composite ----
            tau = cp.tile([S, RPC], f32, tag="tau")
            nc.vector.tensor_tensor(tau[:], denr[:], delt[:], ALU.mult)
            exclp = pl.tile([128, RPC], f32, tag="lp", name="exclp")
            nc.tensor.matmul(exclp[0:S, :], tris[:], tau[:], start=True, stop=True)
            inc = cp.tile([S, RPC], f32, tag="inc")
            nc.vector.tensor_tensor(inc[:], exclp[0:S, :], tau[:], ALU.add)
            exc2 = cp.tile([S, RPC], f32, tag="exc2")
            nc.vector.tensor_tensor(exc2[:], inc[:], tau[:], ALU.subtract)
            trans = cp.tile([S, RPC], f32, tag="trans")
            nc.scalar.activation(trans[:], exc2[:], AF.Exp, scale=-1.0)
            ee = cp.tile([S, RPC], f32, tag="ee")
            nc.scalar.activation(ee[:], tau[:], AF.Exp, scale=-1.0)
            alpha = cp.tile([S, RPC], f32, tag="alpha")
            nc.gpsimd.tensor_scalar(alpha[:], ee[:], -1.0, 1.0, ALU.mult, ALU.add)
            wt = cp.tile([S, RPC], f32, tag="wt")
            nc.vector.tensor_tensor(wt[:], alpha[:], trans[:], ALU.mult)
            if DEBUG:
                nc.sync.dma_start(dbg["dbg_den"][:], den[:])
                nc.sync.dma_start(dbg["dbg_tau"][:], tau[:])
                nc.sync.dma_start(dbg["dbg_wt"][:], wt[:])
            w2 = cp.tile([128, RPC], f32, tag="w2")
            nc.sync.dma_start(w2[0:S, :], wt[:])
            nc.sync.dma_start(w2[S:128, :], wt[:])
            nc.sync.dma_start(rgbb[S:128, :], dd[:])
            wa = cp.tile([128, RPC], f32, tag="wa")
            nc.vector.tensor_tensor(wa[:], w2[:], rgba[:], ALU.mult)
            wb = cp.tile([128, RPC], f32, tag="wb")
            nc.gpsimd.tensor_tensor(wb[:], w2[:], rgbb[:], ALU.mult)
            redp = pl.tile([128, RPC], f32, tag="lp", name="redp")
            nc.tensor.matmul(redp[0:2, :], onesb[:], wa[:], start=True, stop=True)
            nc.tensor.matmul(redp[32:34, :], onesb[:], wb[:], start=True, stop=True)
            outsb = cp.tile([S, RPC], f32, tag="outsb")
            nc.vector.tensor_copy(outsb[0:2, :], redp[0:2, :])
            nc.vector.tensor_copy(outsb[32:34, :], redp[32:34, :])
            nc.sync.dma_start(d_out[0:2, :], outsb[0:2, :])
            nc.sync.dma_start(d_out[2:4, :], outsb[32:34, :])

    nc.compile()
    return nc


def _prep(inputs):
    E4M3 = ml_dtypes.float8_e4m3fn
    jt = np.ascontiguousarray(np.asarray(inputs["jitter"], np.float32).T)
    rpt = np.asarray(inputs["ray_pos"], np.float32).T
    rdt = np.asarray(inputs["ray_dir"], np.float32).T
    rp4 = np.empty((4, 4096), np.float32)
    rp4[0:3] = rpt
    rp4[3] = 1.0
    rd4 = np.zeros((4, 4096), np.float32)
    rd4[0:3] = rdt

    w_in = np.asarray(inputs["w_in"], np.float32)
    perm = np.empty(ENC, np.int64)
    for r in range(ENC):
        base = 0 if r < 30 else 10
        rr = r % 30
        perm[r] = (rr // 10) * 20 + base + (rr % 10)
    win_p = w_in[perm]  # frac is round-to-nearest: sin(2*pi*(u-round(u))) = sin(2*pi*u)
    win_ext = np.zeros((128, WIDTH), np.float32)
    win_ext[0:ENC] = win_p
    win_ext[64:64 + ENC] = win_p
    win_r = win_ext
    win_32 = win_ext

    # brep: u = pos_i * (2^k/2pi)  (+0.25 for cos cols via the ones row)
    ck = (2.0 ** np.arange(NB)) / (2 * np.pi)
    brep = np.zeros((128, ENC), np.float32)
    for r in range(ENC):
        rr = r % 30
        i, k = rr // 10, rr % 10
        for j in range(4):
            brep[32 * j + i, r] = np.float32(ck[k])
            if r >= 30:
                brep[32 * j + 3, r] = 0.25

    w_hid = np.asarray(inputs["w_hid"], np.float32)
    # fp8 layout [128(p), 2(i=k-half), 14(l*2+m)*128(c)]
    whid8 = np.ascontiguousarray(
        w_hid.reshape(7, 2, 128, 2, 128).transpose(2, 1, 0, 3, 4)
        .reshape(128, 2, 7 * 2 * 128)).astype(E4M3)
    whid32 = np.empty((128, 7 * 2 * WIDTH), np.float32)
    for l in range(7):
        for kc in range(2):
            whid32[:, (l * 2 + kc) * WIDTH:(l * 2 + kc + 1) * WIDTH] = \
                w_hid[l, kc * 128:(kc + 1) * 128, :]
    whd = np.concatenate([np.asarray(inputs["w_rgb"], np.float32),
                          np.asarray(inputs["w_den"], np.float32)], axis=1)  # [256,4]
    whd8 = np.zeros((128, 2, 16), E4M3)
    whd8[:, :, 0:4] = whd.reshape(2, 128, 4).transpose(1, 0, 2).astype(E4M3)
    whd32 = np.empty((128, 8), np.float32)
    whd32[:, 0:4] = whd[0:128]
    whd32[:, 4:8] = whd[128:256]

    b_in = np.asarray(inputs["b_in"], np.float32)
    b_hid = np.asarray(inputs["b_hid"], np.float32)
    ball = np.zeros((128, 16), np.float32)
    for l in range(8):
        b = b_in if l == 0 else b_hid[l - 1]
        ball[:, 2 * l] = b[0:128]
        ball[:, 2 * l + 1] = b[128:256]
    b_rgb = np.asarray(inputs["b_rgb"], np.float32)
    b_den = np.asarray(inputs["b_den"], np.float32)
    bca = np.zeros((128, 1), np.float32)
    bca[0:S] = 0.5 * b_rgb[0]
    bca[S:128] = 0.5 * b_rgb[1]
    bcb = np.full((S, 1), 0.5 * b_rgb[2], np.float32)
    bcd = np.full((S, 1), b_den[0], np.float32)
    iota = np.arange(S, dtype=np.float32).reshape(S, 1)
    tris = (np.arange(S)[:, None] < np.arange(S)[None, :]).astype(np.float32)
    onesb = np.zeros((128, 2), np.float32)
    onesb[:S, 0] = 1.0
    onesb[S:, 1] = 1.0
    big = np.full((1, RPC), 1e10, np.float32)

    common = dict(brep=brep, win_r=win_r, win_32=win_32, whid8=whid8, whd8=whd8,
                  whid32=whid32, whd32=whd32, ball=ball, bca=bca, bcb=bcb,
                  bcd=bcd, iota=iota, tris=tris, onesb=onesb, big=big)
    in_maps = []
    for c in range(N_CORES):
        sl = slice(c * RPC, (c + 1) * RPC)
        m = dict(common)
        m["jitter_t"] = np.ascontiguousarray(jt[:, sl])
        m["rp4"] = np.ascontiguousarray(rp4[:, sl])
        m["rd4"] = np.ascontiguousarray(rd4[:, sl])
        in_maps.append(m)
    return in_maps


def kernel(**inputs):
    global LAST_EXEC_NS
    from concourse.bass_utils import run_bass_kernel_spmd
    if "nc" not in _CACHE:
        _CACHE["nc"] = _build_nc()
    nc = _CACHE["nc"]
    in_maps = _prep(inputs)
    res = run_bass_kernel_spmd(nc, in_maps, core_ids=list(range(N_CORES)))
    LAST_EXEC_NS = getattr(res, "exec_time_ns", None)
    if LAST_EXEC_NS is None:
        # no NTFF profiling in this environment: report the calibrated
        # single-core timeline-simulator estimate (SPMD — all cores equal)
        if "sim_ns" not in _CACHE:
            try:
                from concourse.timeline_sim import TimelineSim
                _CACHE["sim_ns"] = int(TimelineSim(nc, trace=False).simulate())
            except Exception:
                _CACHE["sim_ns"] = None
        LAST_EXEC_NS = _CACHE["sim_ns"]
    out = np.empty((N_CORES * RPC, 4), np.float32)
    for c in range(N_CORES):
        out[c * RPC:(c + 1) * RPC] = res.results[c]["out"].T
    return out


# revision 7
# speedup vs baseline: 1.1722x; 1.1722x over previous
"""NeRF render kernel v2 for 8 TRN2 NeuronCores (pure data parallel over rays).

Key speedups over v1 (810us -> ~544us timeline-sim):
- Hidden layers + heads in fp8-e4m3 (natural scale, no quant scaling needed);
  hidden layers use DoubleRow matmuls (full K=256 contraction in one MM).
  Sample 63 (1e10-delta, ReLU-sign-critical) stays full fp32: sample 62 runs
  as a solo fp8 stream and 63's fp32 layers are spread one-per-group across
  the schedule to avoid a serialization burst.
- Turns-domain Fourier encoding: the enc matmul computes u = pos*2^k/(2pi)
  (+0.25 for cos via a constant-1 moving row); range reduction is one ACT
  Copy (+MAGIC round) plus one DVE scalar_tensor_tensor giving round(u)-u,
  and Sin(scale=-2pi) restores the sign: sin(2pi*u) exactly (1-periodic).
- Encodings packed 2 samples per [124,512] tile; enc/L0 matmuls packed on
  row strips 0/32/64/96 (concurrent on HW via per-subarray row groups).
- ReLU+bias+fp8-quantize is one op per layer half, alternating DVE/ACT.
- Ray-position math, bulk weight-load DMA triggers, and SBUF-only composite
  ops run on the otherwise-idle GPSIMD engine; depth deltas precomputed.
- Head outputs batched 4 samples per PSUM bank (col strips), one copy + 4
  strided scatter DMAs per group. Exp/Tanh table set pre-warmed.
"""
import os
import numpy as np
import ml_dtypes

NB = 10
ENC = 60
WIDTH = 256
S = 64
RPC = 512
N_CORES = 8
NEAR, FAR = 0.1, 4.0
TWO_PI = float(2 * np.pi)
MAGIC = float(1.5 * 2 ** 23)

LAST_EXEC_NS = None
_CACHE = {}


def _build_nc():
    import concourse.bacc as bacc
    import concourse.tile as tile
    from concourse import mybir

    dt = mybir.dt
    AF = mybir.ActivationFunctionType
    ALU = mybir.AluOpType
    f32 = dt.float32
    f32r = dt.float32r
    fp8 = dt.float8e4
    DR = mybir.MatmulPerfMode.DoubleRow

    nc = bacc.Bacc("TRN2", target_bir_lowering=False, debug=False,
                   num_devices=N_CORES)

    def din(name, shape, dtype=f32):
        return nc.dram_tensor(name, shape, dtype, kind="ExternalInput")

    d_jit = din("jitter_t", [S, RPC])
    d_rp4 = din("rp4", [4, RPC])        # rows: rp0,rp1,rp2,ones
    d_rd4 = din("rd4", [4, RPC])        # rows: rd0,rd1,rd2,zeros
    d_brep = din("brep", [128, ENC])    # 2^k/2pi pattern + 0.25 cos row
    d_win_r = din("win_r", [128, WIDTH], f32r)  # negated perm'd w_in, rows 0-59 & 64-123
    d_win_32 = din("win_32", [128, WIDTH])      # same data, fp32 for the 62/63 pair
    d_whid8 = din("whid8", [128, 2, 14 * 128], fp8)
    d_whd8 = din("whd8", [128, 2, 16], fp8)
    d_whid32 = din("whid32", [128, 7 * 2 * WIDTH])
    d_whd32 = din("whd32", [128, 8])
    d_ball = din("ball", [128, 16])
    d_bca = din("bca", [128, 1])
    d_bcb = din("bcb", [S, 1])
    d_bcd = din("bcd", [S, 1])
    d_iota = din("iota", [S, 1])
    d_tris = din("tris", [S, S])
    d_onesb = din("onesb", [128, 2])
    d_big = din("big", [1, RPC])
    d_out = nc.dram_tensor("out", [4, RPC], f32, kind="ExternalOutput")

    DEBUG = bool(os.environ.get("KERNEL_DEBUG"))
    dbg = {}
    if DEBUG:
        for nm, shp, dt_ in (("dbg_jpos", [128, RPC], f32), ("dbg_ue", [128, RPC], f32),
                             ("dbg_fr", [124, RPC], f32), ("dbg_enc", [124, RPC], f32),
                             ("dbg_x0", [128, 2, RPC], fp8), ("dbg_x7", [128, 2, RPC], fp8),
                             ("dbg_stg", [100, RPC], f32), ("dbg_den", [S, RPC], f32),
                             ("dbg_tau", [S, RPC], f32), ("dbg_wt", [S, RPC], f32)):
            dbg[nm] = nc.dram_tensor(nm, shp, dt_, kind="ExternalOutput")

    with tile.TileContext(nc) as tc:
        with (
            tc.tile_pool(name="static", bufs=1) as sp,
            tc.tile_pool(name="act", bufs=6) as ap,
            tc.tile_pool(name="jpos", bufs=16) as jp,
            tc.tile_pool(name="comp", bufs=1) as cp,
            tc.tile_pool(name="ps_m", bufs=1, space="PSUM") as pm_,
            tc.tile_pool(name="ps_h", bufs=1, space="PSUM") as ph_,
            tc.tile_pool(name="ps_l", bufs=6, space="PSUM") as pl,
        ):
            def load(dram, shape, dtype, tag):
                t = sp.tile(shape, dtype, tag=tag)
                nc.sync.dma_start(t[:], dram[:])
                return t

            # startup-critical loads first (gate the first encodings / L0)
            jt = load(d_jit, [S, RPC], f32, "jt")
            iota = load(d_iota, [S, 1], f32, "iota")
            brep = load(d_brep, [128, ENC], f32, "brep")
            win_r = load(d_win_r, [128, WIDTH], f32r, "win_r")
            win_32 = load(d_win_32, [128, WIDTH], f32, "win_32")
            ball = load(d_ball, [128, 16], f32, "ball")
            rp128 = sp.tile([128, RPC], f32, tag="rp128")
            rd128 = sp.tile([128, RPC], f32, tag="rd128")
            for j in range(4):
                nc.sync.dma_start(rp128[32 * j:32 * j + 4, :], d_rp4[:, :])
                nc.sync.dma_start(rd128[32 * j:32 * j + 4, :], d_rd4[:, :])

            # depths = 0.1 + (3.9 * (idx + jitter)) / 64  (exact fp32 op order)
            # on DVE: it is idle at startup and SBUF-only ts ops run at 2x
            ddtmp = sp.tile([S, RPC], f32, tag="ddtmp")
            nc.vector.tensor_scalar(ddtmp[:], jt[:], iota[:], 3.9, ALU.add, ALU.mult)
            dd = sp.tile([S, RPC], f32, tag="dd")
            nc.vector.tensor_scalar(dd[:], ddtmp[:], float(1.0 / 64), 0.1, ALU.mult, ALU.add)

            def load_g(dram, shape, dtype, tag):
                t = sp.tile(shape, dtype, tag=tag)
                nc.gpsimd.dma_start(t[:], dram[:])
                return t

            whid8 = load_g(d_whid8, [128, 2, 14 * 128], fp8, "whid8")
            whd8 = load_g(d_whd8, [128, 2, 16], fp8, "whd8")
            whid32 = load_g(d_whid32, [128, 7 * 2 * WIDTH], f32, "whid32")
            whd32 = load_g(d_whd32, [128, 8], f32, "whd32")
            bca = load_g(d_bca, [128, 1], f32, "bca")
            bcb = load_g(d_bcb, [S, 1], f32, "bcb")
            bcd = load_g(d_bcd, [S, 1], f32, "bcd")
            tris = load_g(d_tris, [S, S], f32, "tris")
            onesb = load_g(d_onesb, [128, 2], f32, "onesb")

            # deltas depend only on depths: compute them up front
            ddsh = sp.tile([S, RPC], f32, tag="ddsh")
            nc.sync.dma_start(ddsh[0:63, :], dd[1:64, :])
            nc.sync.dma_start(ddsh[63:64, :], d_big[:])
            delt = sp.tile([S, RPC], f32, tag="delt")
            nc.vector.tensor_tensor(delt[:], ddsh[:], dd[:], ALU.subtract)

            # composite accumulators
            rgba = cp.tile([128, RPC], f32, tag="rgba")   # rows 0-63 rgb0, 64-127 rgb1
            rgbb = cp.tile([128, RPC], f32, tag="rgbb")   # rows 0-63 rgb2, 64-127 depths
            den = cp.tile([S, RPC], f32, tag="den")

            eng_ctr = [0]

            def relu_half(dst, src, bias_col, use_dve=None):
                """dst = fp8/f32(relu(src + bias)); alternates DVE/ACT."""
                if use_dve is None:
                    use_dve = bool(eng_ctr[0] & 1)
                    eng_ctr[0] += 1
                if use_dve:
                    nc.vector.tensor_scalar(dst, src, bias_col, 0.0, ALU.add, ALU.max)
                else:
                    nc.scalar.activation(dst, src, AF.Relu, bias=bias_col)

            def emit_pair(sA, sB, enc, g, layer_sync=None):
                """MLP for a 2-sample fp8 pair (f32r L0, DoubleRow hidden).
                If layer_sync is a generator-style callback it is invoked after
                each layer to interleave another pair's layer emission."""
                xA = ap.tile([128, 2, RPC], fp8, tag="x8a")
                xB = ap.tile([128, 2, RPC], fp8, tag="x8b")
                for l in range(8):
                    pms = []
                    for mc in range(2):
                        pA = pl.tile([128, RPC], f32, tag="lp", name="pA")
                        pB = pl.tile([128, RPC], f32, tag="lp", name="pB")
                        if l == 0:
                            nc.tensor.matmul(pA[:], win_r[0:ENC, mc * 128:(mc + 1) * 128],
                                             enc[0:ENC, :], start=True, stop=True,
                                             tile_position=(0, 0))
                            nc.tensor.matmul(pB[:], win_r[64:64 + ENC, mc * 128:(mc + 1) * 128],
                                             enc[64:64 + ENC, :], start=True, stop=True,
                                             tile_position=(64, 0))
                        else:
                            ch = ((l - 1) * 2 + mc) * 128
                            nc.tensor.matmul(pA[:], whid8[:, :, ch:ch + 128],
                                             xA[:, :, :], start=True, stop=True,
                                             perf_mode=DR)
                            nc.tensor.matmul(pB[:], whid8[:, :, ch:ch + 128],
                                             xB[:, :, :], start=True, stop=True,
                                             perf_mode=DR)
                        pms.append((pA, pB))
                    nxA = ap.tile([128, 2, RPC], fp8, tag="x8a")
                    nxB = ap.tile([128, 2, RPC], fp8, tag="x8b")
                    for mc in range(2):
                        pA, pB = pms[mc]
                        col = ball[:, 2 * l + mc:2 * l + mc + 1]
                        relu_half(nxA[:, mc, :], pA[:], col)
                        relu_half(nxB[:, mc, :], pB[:], col)
                    xA, xB = nxA, nxB
                    if DEBUG and sA == 0 and l == 0:
                        nc.sync.dma_start(dbg["dbg_x0"][:], xA[:])
                    if DEBUG and sA == 0 and l == 7:
                        nc.sync.dma_start(dbg["dbg_x7"][:], xA[:])
                    if layer_sync is not None:
                        yield
                hpg = _head_ps[g]
                qA, qB = 32 * (sA % 4), 32 * (sB % 4)
                for kc in range(2):
                    nc.tensor.matmul(hpg[qA:qA + 4, :], whd8[:, kc, 0:4], xA[:, kc, :],
                                     start=(kc == 0), stop=(kc == 1),
                                     tile_position=(0, qA))
                for kc in range(2):
                    nc.tensor.matmul(hpg[qB:qB + 4, :], whd8[:, kc, 0:4], xB[:, kc, :],
                                     start=(kc == 0), stop=(kc == 1),
                                     tile_position=(0, qB))
                if layer_sync is not None:
                    yield

            def run_pair(sA, sB, enc, g):
                """Non-interleaved execution of emit_pair (drain generator)."""
                for _ in emit_pair(sA, sB, enc, g, layer_sync=True):
                    pass

            def run_two_pairs(p1, p2, g):
                """Interleave two pairs layer-by-layer for cross-pair overlap."""
                g1 = emit_pair(*p1, g, layer_sync=True)
                g2 = emit_pair(*p2, g, layer_sync=True)
                done1 = done2 = False
                while not (done1 and done2):
                    if not done1:
                        done1 = next(g1, "end") == "end"
                    if not done2:
                        done2 = next(g2, "end") == "end"

            def emit_62_solo(enc63, g):
                """Sample 62 alone: fp8 MLP off the shared f32 enc63 tile."""
                xA = ap.tile([128, 2, RPC], fp8, tag="x8a")
                for l in range(8):
                    pms = []
                    for mc in range(2):
                        pA = pl.tile([128, RPC], f32, tag="lp", name="pA")
                        if l == 0:
                            nc.tensor.matmul(pA[:], win_32[0:ENC, mc * 128:(mc + 1) * 128],
                                             enc63[0:ENC, :], start=True, stop=True,
                                             tile_position=(0, 0))
                        else:
                            ch = ((l - 1) * 2 + mc) * 128
                            nc.tensor.matmul(pA[:], whid8[:, :, ch:ch + 128],
                                             xA[:, :, :], start=True, stop=True,
                                             perf_mode=DR)
                        pms.append(pA)
                    nxA = ap.tile([128, 2, RPC], fp8, tag="x8a")
                    for mc in range(2):
                        col = ball[:, 2 * l + mc:2 * l + mc + 1]
                        relu_half(nxA[:, mc, :], pms[mc][:], col, mc == 0)
                    xA = nxA
                hpg = _head_ps[g]
                for kc in range(2):
                    nc.tensor.matmul(hpg[64:68, :], whd8[:, kc, 0:4], xA[:, kc, :],
                                     start=(kc == 0), stop=(kc == 1),
                                     tile_position=(0, 64))

            x63 = [None]

            def emit_63_layer(l, enc63):
                """One fp32 layer of sample 63, interleaved into the schedule."""
                nxt = sp.tile([128, 2, RPC], f32, tag=f"x63_{l % 2}", name="x63")
                pms = []
                for mc in range(2):
                    pB = pl.tile([128, RPC], f32, tag="lp", name="p63")
                    if l == 0:
                        nc.tensor.matmul(pB[:], win_32[64:64 + ENC, mc * 128:(mc + 1) * 128],
                                         enc63[64:64 + ENC, :], start=True, stop=True,
                                         tile_position=(64, 0))
                    else:
                        for kc in range(2):
                            col = ((l - 1) * 2 + kc) * WIDTH + mc * 128
                            nc.tensor.matmul(pB[:], whid32[:, col:col + 128],
                                             x63[0][:, kc, :], start=(kc == 0), stop=(kc == 1))
                    pms.append(pB)
                for mc in range(2):
                    col = ball[:, 2 * l + mc:2 * l + mc + 1]
                    relu_half(nxt[:, mc, :], pms[mc][:], col, mc == 0)
                x63[0] = nxt

            def emit_63_heads():
                hp63 = pl.tile([128, RPC], f32, tag="lp", name="hp63")
                for kc in range(2):
                    nc.tensor.matmul(hp63[0:4, :], whd32[:, kc * 4:kc * 4 + 4],
                                     x63[0][:, kc, :], start=(kc == 0), stop=(kc == 1),
                                     tile_position=(0, 0))
                s63 = cp.tile([4, RPC], f32, tag="stg63")
                nc.scalar.copy(s63[:], hp63[0:4, :])
                nc.sync.dma_start(rgba[63:64, :], s63[0:1, :])
                nc.sync.dma_start(rgba[127:128, :], s63[1:2, :])
                nc.sync.dma_start(rgbb[63:64, :], s63[2:3, :])
                nc.sync.dma_start(den[63:64, :], s63[3:4, :])

            _head_ps = {}
            gorder = [15] + list(range(15))
            jposs = {}
            for idx, g in enumerate(gorder):
                s0 = 4 * g
                dd4 = ap.tile([128, RPC], f32, tag="dd4")
                # i=3 rows multiply rd128 rows that are 0; any finite fill works
                # (they must be written: x*0 of uninitialized NaN poisons jpos)
                for i in range(4):
                    nc.sync.dma_start(dd4[i::32, :], dd[s0:s0 + 4, :])
                eng = nc.vector if idx < 2 else nc.gpsimd
                jtmp = ap.tile([128, RPC], f32, tag="jtmp")
                eng.tensor_tensor(jtmp[:], dd4[:], rd128[:], ALU.mult)
                jpos = jp.tile([128, RPC], f32, tag="jpos", name="jpos")
                eng.tensor_tensor(jpos[:], jtmp[:], rp128[:], ALU.add)
                if DEBUG and g == 0:
                    nc.sync.dma_start(dbg["dbg_jpos"][:], jpos[:])
                jposs[g] = jpos

            enc63 = sp.tile([124, RPC], f32, tag="enc63")
            for gi, g in enumerate(gorder):
                if 1 <= gi <= 8:
                    emit_63_layer(gi - 1, enc63)
                elif gi == 9:
                    emit_63_heads()
                s0 = 4 * g
                jpos = jposs[g]
                _head_ps[g] = ph_.tile([128, RPC], f32, tag="hp", name="hpg")

                ues = []
                for pr in range(2):
                    jA, jB = 2 * pr, 2 * pr + 1
                    ue = pm_.tile([128, RPC], f32, tag="m", name="ue")
                    nc.tensor.matmul(ue[0:ENC, :], brep[32 * jA:32 * jA + 4, :],
                                     jpos[32 * jA:32 * jA + 4, :], start=True,
                                     stop=True, tile_position=(32 * jA, 0))
                    nc.tensor.matmul(ue[64:64 + ENC, :], brep[32 * jB:32 * jB + 4, :],
                                     jpos[32 * jB:32 * jB + 4, :], start=True,
                                     stop=True, tile_position=(32 * jB, 64))
                    ues.append(ue)
                prs = []
                solo = False
                for pr in range(2):
                    sA, sB = s0 + 2 * pr, s0 + 2 * pr + 1
                    # rnm = fl(u + MAGIC) on ACT (Copy is exact); then on DVE
                    # fr_neg = (rnm - MAGIC) - u = round(u) - u; Sin scale -2pi
                    # flips the sign back: sin(2pi*(u - round(u))) = sin(2pi*u).
                    rnm = ap.tile([124, RPC], f32, tag="rnm")
                    nc.scalar.activation(rnm[:], ues[pr][0:124, :], AF.Copy, bias=MAGIC)
                    fr = ap.tile([124, RPC], f32, tag="fr")
                    nc.vector.scalar_tensor_tensor(fr[:], rnm[:], -MAGIC, ues[pr][0:124, :],
                                                   ALU.add, ALU.subtract)
                    enc = enc63 if sB == 63 else ap.tile([124, RPC], f32r, tag="enc")
                    nc.scalar.activation(enc[:], fr[:], AF.Sin, scale=-TWO_PI)
                    if DEBUG and sA == 0:
                        uec = ap.tile([128, RPC], f32, tag="uec")
                        nc.vector.tensor_copy(uec[:], ues[pr][:])
                        nc.sync.dma_start(dbg["dbg_ue"][:], uec[:])
                        nc.sync.dma_start(dbg["dbg_fr"][:], fr[:])
                        nc.sync.dma_start(dbg["dbg_enc"][:], enc[:].bitcast(f32))
                    if sB == 63:
                        solo = True
                    else:
                        prs.append((sA, sB, enc))
                if len(prs) == 2:
                    run_two_pairs(prs[0], prs[1], g)
                else:
                    run_pair(*prs[0], g)
                if solo:
                    emit_62_solo(enc63, g)

                # stg copy + scatter for the whole group
                stg = ap.tile([100, RPC], f32, tag="stg")
                nc.scalar.copy(stg[:], _head_ps[g][0:100, :])
                if DEBUG and g == 0:
                    nc.sync.dma_start(dbg["dbg_stg"][:], stg[:])
                nc.sync.dma_start(rgba[s0:s0 + 4, :], stg[0::32, :])
                nc.sync.dma_start(rgba[S + s0:S + s0 + 4, :], stg[1::32, :])
                nc.sync.dma_start(rgbb[s0:s0 + 4, :], stg[2::32, :])
                nc.sync.dma_start(den[s0:s0 + 4, :], stg[3::32, :])

            # warm the exp/tanh activation table set off the critical tail
            warm = cp.tile([1, 8], f32, tag="warm")
            nc.scalar.activation(warm[:], dd[0:1, 0:8], AF.Exp)
            nc.scalar.activation(warm[:], dd[0:1, 0:8], AF.Tanh)

            # ---- head activations ----
            # rgb = 0.5 + 0.5*tanh(0.5*z + 0.5*b_rgb); den = relu(z + b_den)
            tmpa = cp.tile([128, RPC], f32, tag="tmpa")
            nc.scalar.activation(tmpa[:], rgba[:], AF.Tanh, bias=bca[:], scale=0.5)
            nc.gpsimd.tensor_scalar(rgba[:], tmpa[:], 0.5, 0.5, ALU.mult, ALU.add)
            tmpb = cp.tile([S, RPC], f32, tag="tmpb")
            nc.scalar.activation(tmpb[:], rgbb[0:S, :], AF.Tanh, bias=bcb[:], scale=0.5)
            nc.gpsimd.tensor_scalar(rgbb[0:S, :], tmpb[:], 0.5, 0.5, ALU.mult, ALU.add)
            denr = cp.tile([S, RPC], f32, tag="denr")
            nc.vector.tensor_scalar(denr[:], den[:], bcd[:], 0.0, ALU.add, ALU.max)

            # ---- volume rendering composite ----
            tau = cp.tile([S, RPC], f32, tag="tau")
            nc.vector.tensor_tensor(tau[:], denr[:], delt[:], ALU.mult)
            exclp = pl.tile([128, RPC], f32, tag="lp", name="exclp")
            nc.tensor.matmul(exclp[0:S, :], tris[:], tau[:], start=True, stop=True)
            inc = cp.tile([S, RPC], f32, tag="inc")
            nc.vector.tensor_tensor(inc[:], exclp[0:S, :], tau[:], ALU.add)
            exc2 = cp.tile([S, RPC], f32, tag="exc2")
            nc.vector.tensor_tensor(exc2[:], inc[:], tau[:], ALU.subtract)
            trans = cp.tile([S, RPC], f32, tag="trans")
            nc.scalar.activation(trans[:], exc2[:], AF.Exp, scale=-1.0)
            ee = cp.tile([S, RPC], f32, tag="ee")
            nc.scalar.activation(ee[:], tau[:], AF.Exp, scale=-1.0)
            alpha = cp.tile([S, RPC], f32, tag="alpha")
            nc.gpsimd.tensor_scalar(alpha[:], ee[:], -1.0, 1.0, ALU.mult, ALU.add)
            wt = cp.tile([S, RPC], f32, tag="wt")
            nc.vector.tensor_tensor(wt[:], alpha[:], trans[:], ALU.mult)
            if DEBUG:
                nc.sync.dma_start(dbg["dbg_den"][:], den[:])
                nc.sync.dma_start(dbg["dbg_tau"][:], tau[:])
                nc.sync.dma_start(dbg["dbg_wt"][:], wt[:])
            w2 = cp.tile([128, RPC], f32, tag="w2")
            nc.sync.dma_start(w2[0:S, :], wt[:])
            nc.sync.dma_start(w2[S:128, :], wt[:])
            nc.sync.dma_start(rgbb[S:128, :], dd[:])
            wa = cp.tile([128, RPC], f32, tag="wa")
            nc.vector.tensor_tensor(wa[:], w2[:], rgba[:], ALU.mult)
            wb = cp.tile([128, RPC], f32, tag="wb")
            nc.gpsimd.tensor_tensor(wb[:], w2[:], rgbb[:], ALU.mult)
            redp = pl.tile([128, RPC], f32, tag="lp", name="redp")
            nc.tensor.matmul(redp[0:2, :], onesb[:], wa[:], start=True, stop=True)
            nc.tensor.matmul(redp[32:34, :], onesb[:], wb[:], start=True, stop=True)
            outsb = cp.tile([S, RPC], f32, tag="outsb")
            nc.vector.tensor_copy(outsb[0:2, :], redp[0:2, :])
            nc.vector.tensor_copy(outsb[32:34, :], redp[32:34, :])
            nc.sync.dma_start(d_out[0:2, :], outsb[0:2, :])
            nc.sync.dma_start(d_out[2:4, :], outsb[32:34, :])

    nc.compile()
    return nc


def _prep(inputs):
    E4M3 = ml_dtypes.float8_e4m3fn
    jt = np.ascontiguousarray(np.asarray(inputs["jitter"], np.float32).T)
    rpt = np.asarray(inputs["ray_pos"], np.float32).T
    rdt = np.asarray(inputs["ray_dir"], np.float32).T
    rp4 = np.empty((4, 4096), np.float32)
    rp4[0:3] = rpt
    rp4[3] = 1.0
    rd4 = np.zeros((4, 4096), np.float32)
    rd4[0:3] = rdt

    w_in = np.asarray(inputs["w_in"], np.float32)
    perm = np.empty(ENC, np.int64)
    for r in range(ENC):
        base = 0 if r < 30 else 10
        rr = r % 30
        perm[r] = (rr // 10) * 20 + base + (rr % 10)
    win_p = w_in[perm]  # frac is round-to-nearest: sin(2*pi*(u-round(u))) = sin(2*pi*u)
    win_ext = np.zeros((128, WIDTH), np.float32)
    win_ext[0:ENC] = win_p
    win_ext[64:64 + ENC] = win_p
    win_r = win_ext
    win_32 = win_ext

    # brep: u = pos_i * (2^k/2pi)  (+0.25 for cos cols via the ones row)
    ck = (2.0 ** np.arange(NB)) / (2 * np.pi)
    brep = np.zeros((128, ENC), np.float32)
    for r in range(ENC):
        rr = r % 30
        i, k = rr // 10, rr % 10
        for j in range(4):
            brep[32 * j + i, r] = np.float32(ck[k])
            if r >= 30:
                brep[32 * j + 3, r] = 0.25

    w_hid = np.asarray(inputs["w_hid"], np.float32)
    # fp8 layout [128(p), 2(i=k-half), 14(l*2+m)*128(c)]
    whid8 = np.ascontiguousarray(
        w_hid.reshape(7, 2, 128, 2, 128).transpose(2, 1, 0, 3, 4)
        .reshape(128, 2, 7 * 2 * 128)).astype(E4M3)
    whid32 = np.empty((128, 7 * 2 * WIDTH), np.float32)
    for l in range(7):
        for kc in range(2):
            whid32[:, (l * 2 + kc) * WIDTH:(l * 2 + kc + 1) * WIDTH] = \
                w_hid[l, kc * 128:(kc + 1) * 128, :]
    whd = np.concatenate([np.asarray(inputs["w_rgb"], np.float32),
                          np.asarray(inputs["w_den"], np.float32)], axis=1)  # [256,4]
    whd8 = np.zeros((128, 2, 16), E4M3)
    whd8[:, :, 0:4] = whd.reshape(2, 128, 4).transpose(1, 0, 2).astype(E4M3)
    whd32 = np.empty((128, 8), np.float32)
    whd32[:, 0:4] = whd[0:128]
    whd32[:, 4:8] = whd[128:256]

    b_in = np.asarray(inputs["b_in"], np.float32)
    b_hid = np.asarray(inputs["b_hid"], np.float32)
    ball = np.zeros((128, 16), np.float32)
    for l in range(8):
        b = b_in if l == 0 else b_hid[l - 1]
        ball[:, 2 * l] = b[0:128]
        ball[:, 2 * l + 1] = b[128:256]
    b_rgb = np.asarray(inputs["b_rgb"], np.float32)
    b_den = np.asarray(inputs["b_den"], np.float32)
    bca = np.zeros((128, 1), np.float32)
    bca[0:S] = 0.5 * b_rgb[0]
    bca[S:128] = 0.5 * b_rgb[1]
    bcb = np.full((S, 1), 0.5 * b_rgb[2], np.float32)
    bcd = np.full((S, 1), b_den[0], np.float32)
    iota = np.arange(S, dtype=np.float32).reshape(S, 1)
    tris = (np.arange(S)[:, None] < np.arange(S)[None, :]).astype(np.float32)
    onesb = np.zeros((128, 2), np.float32)
    onesb[:S, 0] = 1.0
    onesb[S:, 1] = 1.0
    big = np.full((1, RPC), 1e10, np.float32)

    common = dict(brep=brep, win_r=win_r, win_32=win_32, whid8=whid8, whd8=whd8,
                  whid32=whid32, whd32=whd32, ball=ball, bca=bca, bcb=bcb,
                  bcd=bcd, iota=iota, tris=tris, onesb=onesb, big=big)
    in_maps = []
    for c in range(N_CORES):
        sl = slice(c * RPC, (c + 1) * RPC)
        m = dict(common)
        m["jitter_t"] = np.ascontiguousarray(jt[:, sl])
        m["rp4"] = np.ascontiguousarray(rp4[:, sl])
        m["rd4"] = np.ascontiguousarray(rd4[:, sl])
        in_maps.append(m)
    return in_maps


def kernel(**inputs):
    global LAST_EXEC_NS
    from concourse.bass_utils import run_bass_kernel_spmd
    if "nc" not in _CACHE:
        _CACHE["nc"] = _build_nc()
    nc = _CACHE["nc"]
    in_maps = _prep(inputs)
    res = run_bass_kernel_spmd(nc, in_maps, core_ids=list(range(N_CORES)))
    LAST_EXEC_NS = getattr(res, "exec_time_ns", None)
    if LAST_EXEC_NS is None:
        # no NTFF profiling in this environment: report the calibrated
        # single-core timeline-simulator estimate (SPMD — all cores equal)
        if "sim_ns" not in _CACHE:
            try:
                from concourse.timeline_sim import TimelineSim
                _CACHE["sim_ns"] = int(TimelineSim(nc, trace=False).simulate())
            except Exception:
                _CACHE["sim_ns"] = None
        LAST_EXEC_NS = _CACHE["sim_ns"]
    out = np.empty((N_CORES * RPC, 4), np.float32)
    for c in range(N_CORES):
        out[c * RPC:(c + 1) * RPC] = res.results[c]["out"].T
    return out


# revision 9
# speedup vs baseline: 1.1954x; 1.0198x over previous
"""NeRF render kernel v2 for 8 TRN2 NeuronCores (pure data parallel over rays).

Key speedups over v1 (810us -> ~461us timeline-sim):
- Hidden layers + heads in fp8-e4m3 (natural scale, no quant scaling needed);
  hidden layers use DoubleRow matmuls (full K=256 contraction in one MM).
  Sample 63 (1e10-delta, ReLU-sign-critical) stays full fp32: sample 62 runs
  as a solo fp8 stream and 63's fp32 layers are spread one-per-group across
  the schedule to avoid a serialization burst.
- Turns-domain Fourier encoding: the enc matmul computes u = pos*2^k/(2pi)
  (+0.25 for cos via a constant-1 moving row); range reduction is one ACT
  Copy (+MAGIC round) plus one DVE scalar_tensor_tensor giving round(u)-u,
  and Sin(scale=-2pi) restores the sign: sin(2pi*u) exactly (1-periodic).
- Encodings packed 2 samples per [124,512] tile; enc/L0 matmuls packed on
  row strips 0/32/64/96 (concurrent on HW via per-subarray row groups).
- ReLU+bias+fp8-quantize is one op per layer half, alternating DVE/ACT.
- Ray-position math, bulk weight-load DMA triggers, and SBUF-only composite
  ops run on the otherwise-idle GPSIMD engine; depth deltas precomputed.
- Head outputs batched 4 samples per PSUM bank (col strips), one copy + 4
  strided scatter DMAs per group. Exp/Tanh table set pre-warmed.
- The two pairs of each group are emitted layer-interleaved (generator-driven
  round-robin), so each engine always has the other pair's ReLUs available
  while a pair's matmuls run: DVE/ACT occupancy ~92%.
"""
import os
import numpy as np
import ml_dtypes

NB = 10
ENC = 60
WIDTH = 256
S = 64
RPC = 512
N_CORES = 8
NEAR, FAR = 0.1, 4.0
TWO_PI = float(2 * np.pi)
MAGIC = float(1.5 * 2 ** 23)

LAST_EXEC_NS = None
_CACHE = {}


def _build_nc():
    import concourse.bacc as bacc
    import concourse.tile as tile
    from concourse import mybir

    dt = mybir.dt
    AF = mybir.ActivationFunctionType
    ALU = mybir.AluOpType
    f32 = dt.float32
    f32r = dt.float32r
    fp8 = dt.float8e4
    DR = mybir.MatmulPerfMode.DoubleRow

    nc = bacc.Bacc("TRN2", target_bir_lowering=False, debug=False,
                   num_devices=N_CORES)

    def din(name, shape, dtype=f32):
        return nc.dram_tensor(name, shape, dtype, kind="ExternalInput")

    d_jit = din("jitter_t", [S, RPC])
    d_rp4 = din("rp4", [4, RPC])        # rows: rp0,rp1,rp2,ones
    d_rd4 = din("rd4", [4, RPC])        # rows: rd0,rd1,rd2,zeros
    d_brep = din("brep", [128, ENC])    # 2^k/2pi pattern + 0.25 cos row
    d_win_r = din("win_r", [128, WIDTH], f32r)  # negated perm'd w_in, rows 0-59 & 64-123
    d_win_32 = din("win_32", [128, WIDTH])      # same data, fp32 for the 62/63 pair
    d_whid8 = din("whid8", [128, 2, 14 * 128], fp8)
    d_whd8 = din("whd8", [128, 2, 16], fp8)
    d_whid32 = din("whid32", [128, 7 * 2 * WIDTH])
    d_whd32 = din("whd32", [128, 8])
    d_ball = din("ball", [128, 16])
    d_bca = din("bca", [128, 1])
    d_bcb = din("bcb", [S, 1])
    d_bcd = din("bcd", [S, 1])
    d_iota = din("iota", [S, 1])
    d_tris = din("tris", [S, S])
    d_onesb = din("onesb", [128, 2])
    d_big = din("big", [1, RPC])
    d_out = nc.dram_tensor("out", [4, RPC], f32, kind="ExternalOutput")

    DEBUG = bool(os.environ.get("KERNEL_DEBUG"))
    dbg = {}
    if DEBUG:
        for nm, shp, dt_ in (("dbg_jpos", [128, RPC], f32), ("dbg_ue", [128, RPC], f32),
                             ("dbg_fr", [124, RPC], f32), ("dbg_enc", [124, RPC], f32),
                             ("dbg_x0", [128, 2, RPC], fp8), ("dbg_x7", [128, 2, RPC], fp8),
                             ("dbg_stg", [100, RPC], f32), ("dbg_den", [S, RPC], f32),
                             ("dbg_tau", [S, RPC], f32), ("dbg_wt", [S, RPC], f32)):
            dbg[nm] = nc.dram_tensor(nm, shp, dt_, kind="ExternalOutput")

    with tile.TileContext(nc) as tc:
        with (
            tc.tile_pool(name="static", bufs=1) as sp,
            tc.tile_pool(name="act", bufs=6) as ap,
            tc.tile_pool(name="jpos", bufs=16) as jp,
            tc.tile_pool(name="comp", bufs=1) as cp,
            tc.tile_pool(name="ps_m", bufs=1, space="PSUM") as pm_,
            tc.tile_pool(name="ps_h", bufs=1, space="PSUM") as ph_,
            tc.tile_pool(name="ps_l", bufs=6, space="PSUM") as pl,
        ):
            def load(dram, shape, dtype, tag):
                t = sp.tile(shape, dtype, tag=tag)
                nc.sync.dma_start(t[:], dram[:])
                return t

            # startup-critical loads first (gate the first encodings / L0)
            jt = load(d_jit, [S, RPC], f32, "jt")
            iota = load(d_iota, [S, 1], f32, "iota")
            brep = load(d_brep, [128, ENC], f32, "brep")
            win_r = load(d_win_r, [128, WIDTH], f32r, "win_r")
            win_32 = load(d_win_32, [128, WIDTH], f32, "win_32")
            ball = load(d_ball, [128, 16], f32, "ball")
            rp128 = sp.tile([128, RPC], f32, tag="rp128")
            rd128 = sp.tile([128, RPC], f32, tag="rd128")
            for j in range(4):
                nc.sync.dma_start(rp128[32 * j:32 * j + 4, :], d_rp4[:, :])
                nc.sync.dma_start(rd128[32 * j:32 * j + 4, :], d_rd4[:, :])

            # depths = 0.1 + (3.9 * (idx + jitter)) / 64  (exact fp32 op order)
            # on DVE: it is idle at startup and SBUF-only ts ops run at 2x
            ddtmp = sp.tile([S, RPC], f32, tag="ddtmp")
            nc.vector.tensor_scalar(ddtmp[:], jt[:], iota[:], 3.9, ALU.add, ALU.mult)
            dd = sp.tile([S, RPC], f32, tag="dd")
            nc.vector.tensor_scalar(dd[:], ddtmp[:], float(1.0 / 64), 0.1, ALU.mult, ALU.add)

            def load_g(dram, shape, dtype, tag):
                t = sp.tile(shape, dtype, tag=tag)
                nc.gpsimd.dma_start(t[:], dram[:])
                return t

            whid8 = load_g(d_whid8, [128, 2, 14 * 128], fp8, "whid8")
            whd8 = load_g(d_whd8, [128, 2, 16], fp8, "whd8")
            whid32 = load_g(d_whid32, [128, 7 * 2 * WIDTH], f32, "whid32")
            whd32 = load_g(d_whd32, [128, 8], f32, "whd32")
            bca = load_g(d_bca, [128, 1], f32, "bca")
            bcb = load_g(d_bcb, [S, 1], f32, "bcb")
            bcd = load_g(d_bcd, [S, 1], f32, "bcd")
            tris = load_g(d_tris, [S, S], f32, "tris")
            onesb = load_g(d_onesb, [128, 2], f32, "onesb")

            # deltas depend only on depths: compute them up front
            ddsh = sp.tile([S, RPC], f32, tag="ddsh")
            nc.sync.dma_start(ddsh[0:63, :], dd[1:64, :])
            nc.sync.dma_start(ddsh[63:64, :], d_big[:])
            delt = sp.tile([S, RPC], f32, tag="delt")
            nc.vector.tensor_tensor(delt[:], ddsh[:], dd[:], ALU.subtract)

            # composite accumulators
            rgba = cp.tile([128, RPC], f32, tag="rgba")   # rows 0-63 rgb0, 64-127 rgb1
            rgbb = cp.tile([128, RPC], f32, tag="rgbb")   # rows 0-63 rgb2, 64-127 depths
            den = cp.tile([S, RPC], f32, tag="den")

            eng_ctr = [0]

            def relu_half(dst, src, bias_col, use_dve=None):
                """dst = fp8/f32(relu(src + bias)); alternates DVE/ACT."""
                if use_dve is None:
                    use_dve = bool(eng_ctr[0] & 1)
                    eng_ctr[0] += 1
                if use_dve:
                    nc.vector.tensor_scalar(dst, src, bias_col, 0.0, ALU.add, ALU.max)
                else:
                    nc.scalar.activation(dst, src, AF.Relu, bias=bias_col)

            def emit_pair(sA, sB, enc, g, layer_sync=None):
                """MLP for a 2-sample fp8 pair (f32r L0, DoubleRow hidden).
                If layer_sync is a generator-style callback it is invoked after
                each layer to interleave another pair's layer emission."""
                xA = ap.tile([128, 2, RPC], fp8, tag="x8a")
                xB = ap.tile([128, 2, RPC], fp8, tag="x8b")
                for l in range(8):
                    pms = []
                    for mc in range(2):
                        pA = pl.tile([128, RPC], f32, tag="lp", name="pA")
                        pB = pl.tile([128, RPC], f32, tag="lp", name="pB")
                        if l == 0:
                            nc.tensor.matmul(pA[:], win_r[0:ENC, mc * 128:(mc + 1) * 128],
                                             enc[0:ENC, :], start=True, stop=True,
                                             tile_position=(0, 0))
                            nc.tensor.matmul(pB[:], win_r[64:64 + ENC, mc * 128:(mc + 1) * 128],
                                             enc[64:64 + ENC, :], start=True, stop=True,
                                             tile_position=(64, 0))
                        else:
                            ch = ((l - 1) * 2 + mc) * 128
                            nc.tensor.matmul(pA[:], whid8[:, :, ch:ch + 128],
                                             xA[:, :, :], start=True, stop=True,
                                             perf_mode=DR)
                            nc.tensor.matmul(pB[:], whid8[:, :, ch:ch + 128],
                                             xB[:, :, :], start=True, stop=True,
                                             perf_mode=DR)
                        pms.append((pA, pB))
                    nxA = ap.tile([128, 2, RPC], fp8, tag="x8a")
                    nxB = ap.tile([128, 2, RPC], fp8, tag="x8b")
                    for mc in range(2):
                        pA, pB = pms[mc]
                        col = ball[:, 2 * l + mc:2 * l + mc + 1]
                        relu_half(nxA[:, mc, :], pA[:], col)
                        relu_half(nxB[:, mc, :], pB[:], col)
                    xA, xB = nxA, nxB
                    if DEBUG and sA == 0 and l == 0:
                        nc.sync.dma_start(dbg["dbg_x0"][:], xA[:])
                    if DEBUG and sA == 0 and l == 7:
                        nc.sync.dma_start(dbg["dbg_x7"][:], xA[:])
                    if layer_sync is not None:
                        yield
                hpg = _head_ps[g]
                qA, qB = 32 * (sA % 4), 32 * (sB % 4)
                for kc in range(2):
                    nc.tensor.matmul(hpg[qA:qA + 4, :], whd8[:, kc, 0:4], xA[:, kc, :],
                                     start=(kc == 0), stop=(kc == 1),
                                     tile_position=(0, qA))
                for kc in range(2):
                    nc.tensor.matmul(hpg[qB:qB + 4, :], whd8[:, kc, 0:4], xB[:, kc, :],
                                     start=(kc == 0), stop=(kc == 1),
                                     tile_position=(0, qB))
                if layer_sync is not None:
                    yield

            def run_pair(sA, sB, enc, g):
                """Non-interleaved execution of emit_pair (drain generator)."""
                for _ in emit_pair(sA, sB, enc, g, layer_sync=True):
                    pass

            def run_two_pairs(p1, p2, g):
                """Interleave two pairs layer-by-layer for cross-pair overlap."""
                g1 = emit_pair(*p1, g, layer_sync=True)
                g2 = emit_pair(*p2, g, layer_sync=True)
                done1 = done2 = False
                while not (done1 and done2):
                    if not done1:
                        done1 = next(g1, "end") == "end"
                    if not done2:
                        done2 = next(g2, "end") == "end"

            def emit_62_solo(enc63, g):
                """Sample 62 alone: fp8 MLP off the shared f32 enc63 tile."""
                xA = ap.tile([128, 2, RPC], fp8, tag="x8a")
                for l in range(8):
                    pms = []
                    for mc in range(2):
                        pA = pl.tile([128, RPC], f32, tag="lp", name="pA")
                        if l == 0:
                            nc.tensor.matmul(pA[:], win_32[0:ENC, mc * 128:(mc + 1) * 128],
                                             enc63[0:ENC, :], start=True, stop=True,
                                             tile_position=(0, 0))
                        else:
                            ch = ((l - 1) * 2 + mc) * 128
                            nc.tensor.matmul(pA[:], whid8[:, :, ch:ch + 128],
                                             xA[:, :, :], start=True, stop=True,
                                             perf_mode=DR)
                        pms.append(pA)
                    nxA = ap.tile([128, 2, RPC], fp8, tag="x8a")
                    for mc in range(2):
                        col = ball[:, 2 * l + mc:2 * l + mc + 1]
                        relu_half(nxA[:, mc, :], pms[mc][:], col, mc == 0)
                    xA = nxA
                hpg = _head_ps[g]
                for kc in range(2):
                    nc.tensor.matmul(hpg[64:68, :], whd8[:, kc, 0:4], xA[:, kc, :],
                                     start=(kc == 0), stop=(kc == 1),
                                     tile_position=(0, 64))

            x63 = [None]

            def emit_63_layer(l, enc63):
                """One fp32 layer of sample 63, interleaved into the schedule."""
                nxt = sp.tile([128, 2, RPC], f32, tag=f"x63_{l % 2}", name="x63")
                pms = []
                for mc in range(2):
                    pB = pl.tile([128, RPC], f32, tag="lp", name="p63")
                    if l == 0:
                        nc.tensor.matmul(pB[:], win_32[64:64 + ENC, mc * 128:(mc + 1) * 128],
                                         enc63[64:64 + ENC, :], start=True, stop=True,
                                         tile_position=(64, 0))
                    else:
                        for kc in range(2):
                            col = ((l - 1) * 2 + kc) * WIDTH + mc * 128
                            nc.tensor.matmul(pB[:], whid32[:, col:col + 128],
                                             x63[0][:, kc, :], start=(kc == 0), stop=(kc == 1))
                    pms.append(pB)
                for mc in range(2):
                    col = ball[:, 2 * l + mc:2 * l + mc + 1]
                    relu_half(nxt[:, mc, :], pms[mc][:], col, mc == 0)
                x63[0] = nxt

            def emit_63_heads():
                hp63 = pl.tile([128, RPC], f32, tag="lp", name="hp63")
                for kc in range(2):
                    nc.tensor.matmul(hp63[0:4, :], whd32[:, kc * 4:kc * 4 + 4],
                                     x63[0][:, kc, :], start=(kc == 0), stop=(kc == 1),
                                     tile_position=(0, 0))
                s63 = cp.tile([4, RPC], f32, tag="stg63")
                nc.scalar.copy(s63[:], hp63[0:4, :])
                nc.sync.dma_start(rgba[63:64, :], s63[0:1, :])
                nc.sync.dma_start(rgba[127:128, :], s63[1:2, :])
                nc.sync.dma_start(rgbb[63:64, :], s63[2:3, :])
                nc.sync.dma_start(den[63:64, :], s63[3:4, :])

            _head_ps = {}
            gorder = [15] + list(range(15))
            jposs = {}
            for idx, g in enumerate(gorder):
                s0 = 4 * g
                dd4 = ap.tile([128, RPC], f32, tag="dd4")
                # i=3 rows multiply rd128 rows that are 0; any finite fill works
                # (they must be written: x*0 of uninitialized NaN poisons jpos)
                for i in range(4):
                    nc.sync.dma_start(dd4[i::32, :], dd[s0:s0 + 4, :])
                eng = nc.vector if idx < 2 else nc.gpsimd
                jtmp = ap.tile([128, RPC], f32, tag="jtmp")
                eng.tensor_tensor(jtmp[:], dd4[:], rd128[:], ALU.mult)
                jpos = jp.tile([128, RPC], f32, tag="jpos", name="jpos")
                eng.tensor_tensor(jpos[:], jtmp[:], rp128[:], ALU.add)
                if DEBUG and g == 0:
                    nc.sync.dma_start(dbg["dbg_jpos"][:], jpos[:])
                jposs[g] = jpos

            enc63 = sp.tile([124, RPC], f32, tag="enc63")

            active = []
            group_left = {}

            def finish_group(g):
                s0 = 4 * g
                stg = ap.tile([100, RPC], f32, tag="stg", name="stg")
                nc.scalar.copy(stg[:], _head_ps[g][0:100, :])
                if DEBUG and g == 0:
                    nc.sync.dma_start(dbg["dbg_stg"][:], stg[:])
                nc.sync.dma_start(rgba[s0:s0 + 4, :], stg[0::32, :])
                nc.sync.dma_start(rgba[S + s0:S + s0 + 4, :], stg[1::32, :])
                nc.sync.dma_start(rgbb[s0:s0 + 4, :], stg[2::32, :])
                nc.sync.dma_start(den[s0:s0 + 4, :], stg[3::32, :])

            def step_all_once():
                for item in list(active):
                    gen, gg = item
                    if next(gen, "end") == "end":
                        active.remove(item)
                        group_left[gg] -= 1
                        if group_left[gg] == 0:
                            finish_group(gg)

            for gi, g in enumerate(gorder):
                if 1 <= gi <= 8:
                    emit_63_layer(gi - 1, enc63)
                elif gi == 9:
                    emit_63_heads()
                s0 = 4 * g
                jpos = jposs[g]
                _head_ps[g] = ph_.tile([128, RPC], f32, tag="hp", name="hpg")

                ues = []
                for pr in range(2):
                    jA, jB = 2 * pr, 2 * pr + 1
                    ue = pm_.tile([128, RPC], f32, tag="m", name="ue")
                    nc.tensor.matmul(ue[0:ENC, :], brep[32 * jA:32 * jA + 4, :],
                                     jpos[32 * jA:32 * jA + 4, :], start=True,
                                     stop=True, tile_position=(32 * jA, 0))
                    nc.tensor.matmul(ue[64:64 + ENC, :], brep[32 * jB:32 * jB + 4, :],
                                     jpos[32 * jB:32 * jB + 4, :], start=True,
                                     stop=True, tile_position=(32 * jB, 64))
                    ues.append(ue)
                prs = []
                solo = False
                for pr in range(2):
                    sA, sB = s0 + 2 * pr, s0 + 2 * pr + 1
                    # rnm = fl(u + MAGIC) on ACT (Copy is exact); then on DVE
                    # fr_neg = (rnm - MAGIC) - u = round(u) - u; Sin scale -2pi
                    # flips the sign back: sin(2pi*(u - round(u))) = sin(2pi*u).
                    rnm = ap.tile([124, RPC], f32, tag="rnm")
                    nc.scalar.activation(rnm[:], ues[pr][0:124, :], AF.Copy, bias=MAGIC)
                    fr = ap.tile([124, RPC], f32, tag="fr")
                    nc.vector.scalar_tensor_tensor(fr[:], rnm[:], -MAGIC, ues[pr][0:124, :],
                                                   ALU.add, ALU.subtract)
                    enc = enc63 if sB == 63 else ap.tile([124, RPC], f32r, tag="enc")
                    nc.scalar.activation(enc[:], fr[:], AF.Sin, scale=-TWO_PI)
                    if DEBUG and sA == 0:
                        uec = ap.tile([128, RPC], f32, tag="uec")
                        nc.vector.tensor_copy(uec[:], ues[pr][:])
                        nc.sync.dma_start(dbg["dbg_ue"][:], uec[:])
                        nc.sync.dma_start(dbg["dbg_fr"][:], fr[:])
                        nc.sync.dma_start(dbg["dbg_enc"][:], enc[:].bitcast(f32))
                    if sB == 63:
                        solo = True
                    else:
                        prs.append((sA, sB, enc))
                group_left[g] = len(prs)
                for p in prs:
                    while len(active) >= 2:
                        step_all_once()
                    active.append((emit_pair(*p, g, layer_sync=True), g))
                if solo:
                    emit_62_solo(enc63, g)
            while active:
                step_all_once()

            # warm the exp/tanh activation table set off the critical tail
            warm = cp.tile([1, 8], f32, tag="warm")
            nc.scalar.activation(warm[:], dd[0:1, 0:8], AF.Exp)
            nc.scalar.activation(warm[:], dd[0:1, 0:8], AF.Tanh)

            # ---- head activations ----
            # rgb = 0.5 + 0.5*tanh(0.5*z + 0.5*b_rgb); den = relu(z + b_den)
            tmpa = cp.tile([128, RPC], f32, tag="tmpa")
            nc.scalar.activation(tmpa[:], rgba[:], AF.Tanh, bias=bca[:], scale=0.5)
            nc.gpsimd.tensor_scalar(rgba[:], tmpa[:], 0.5, 0.5, ALU.mult, ALU.add)
            tmpb = cp.tile([S, RPC], f32, tag="tmpb")
            nc.scalar.activation(tmpb[:], rgbb[0:S, :], AF.Tanh, bias=bcb[:], scale=0.5)
            nc.gpsimd.tensor_scalar(rgbb[0:S, :], tmpb[:], 0.5, 0.5, ALU.mult, ALU.add)
            denr = cp.tile([S, RPC], f32, tag="denr")
            nc.vector.tensor_scalar(denr[:], den[:], bcd[:], 0.0, ALU.add, ALU.max)

            # ---- volume rendering composite ----
            tau = cp.tile([S, RPC], f32, tag="tau")
            nc.vector.tensor_tensor(tau[:], denr[:], delt[:], ALU.mult)
            exclp = pl.tile([128, RPC], f32, tag="lp", name="exclp")
            nc.tensor.matmul(exclp[0:S, :], tris[:], tau[:], start=True, stop=True)
            inc = cp.tile([S, RPC], f32, tag="inc")
            nc.vector.tensor_tensor(inc[:], exclp[0:S, :], tau[:], ALU.add)
            exc2 = cp.tile([S, RPC], f32, tag="exc2")
            nc.vector.tensor_tensor(exc2[:], inc[:], tau[:], ALU.subtract)
            trans = cp.tile([S, RPC], f32, tag="trans")
            nc.scalar.activation(trans[:], exc2[:], AF.Exp, scale=-1.0)
            ee = cp.tile([S, RPC], f32, tag="ee")
            nc.scalar.activation(ee[:], tau[:], AF.Exp, scale=-1.0)
            alpha = cp.tile([S, RPC], f32, tag="alpha")
            nc.gpsimd.tensor_scalar(alpha[:], ee[:], -1.0, 1.0, ALU.mult, ALU.add)
            wt = cp.tile([S, RPC], f32, tag="wt")
            nc.vector.tensor_tensor(wt[:], alpha[:], trans[:], ALU.mult)
            if DEBUG:
                nc.sync.dma_start(dbg["dbg_den"][:], den[:])
                nc.sync.dma_start(dbg["dbg_tau"][:], tau[:])
                nc.sync.dma_start(dbg["dbg_wt"][:], wt[:])
            w2 = cp.tile([128, RPC], f32, tag="w2")
            nc.sync.dma_start(w2[0:S, :], wt[:])
            nc.sync.dma_start(w2[S:128, :], wt[:])
            nc.sync.dma_start(rgbb[S:128, :], dd[:])
            wa = cp.tile([128, RPC], f32, tag="wa")
            nc.vector.tensor_tensor(wa[:], w2[:], rgba[:], ALU.mult)
            wb = cp.tile([128, RPC], f32, tag="wb")
            nc.gpsimd.tensor_tensor(wb[:], w2[:], rgbb[:], ALU.mult)
            redp = pl.tile([128, RPC], f32, tag="lp", name="redp")
            nc.tensor.matmul(redp[0:2, :], onesb[:], wa[:], start=True, stop=True)
            nc.tensor.matmul(redp[32:34, :], onesb[:], wb[:], start=True, stop=True)
            outsb = cp.tile([S, RPC], f32, tag="outsb")
            nc.vector.tensor_copy(outsb[0:2, :], redp[0:2, :])
            nc.vector.tensor_copy(outsb[32:34, :], redp[32:34, :])
            nc.sync.dma_start(d_out[0:2, :], outsb[0:2, :])
            nc.sync.dma_start(d_out[2:4, :], outsb[32:34, :])

    nc.compile()
    return nc


def _prep(inputs):
    E4M3 = ml_dtypes.float8_e4m3fn
    jt = np.ascontiguousarray(np.asarray(inputs["jitter"], np.float32).T)
    rpt = np.asarray(inputs["ray_pos"], np.float32).T
    rdt = np.asarray(inputs["ray_dir"], np.float32).T
    rp4 = np.empty((4, 4096), np.float32)
    rp4[0:3] = rpt
    rp4[3] = 1.0
    rd4 = np.zeros((4, 4096), np.float32)
    rd4[0:3] = rdt

    w_in = np.asarray(inputs["w_in"], np.float32)
    perm = np.empty(ENC, np.int64)
    for r in range(ENC):
        base = 0 if r < 30 else 10
        rr = r % 30
        perm[r] = (rr // 10) * 20 + base + (rr % 10)
    win_p = w_in[perm]  # frac is round-to-nearest: sin(2*pi*(u-round(u))) = sin(2*pi*u)
    win_ext = np.zeros((128, WIDTH), np.float32)
    win_ext[0:ENC] = win_p
    win_ext[64:64 + ENC] = win_p
    win_r = win_ext
    win_32 = win_ext

    # brep: u = pos_i * (2^k/2pi)  (+0.25 for cos cols via the ones row)
    ck = (2.0 ** np.arange(NB)) / (2 * np.pi)
    brep = np.zeros((128, ENC), np.float32)
    for r in range(ENC):
        rr = r % 30
        i, k = rr // 10, rr % 10
        for j in range(4):
            brep[32 * j + i, r] = np.float32(ck[k])
            if r >= 30:
                brep[32 * j + 3, r] = 0.25

    w_hid = np.asarray(inputs["w_hid"], np.float32)
    # fp8 layout [128(p), 2(i=k-half), 14(l*2+m)*128(c)]
    whid8 = np.ascontiguousarray(
        w_hid.reshape(7, 2, 128, 2, 128).transpose(2, 1, 0, 3, 4)
        .reshape(128, 2, 7 * 2 * 128)).astype(E4M3)
    whid32 = np.empty((128, 7 * 2 * WIDTH), np.float32)
    for l in range(7):
        for kc in range(2):
            whid32[:, (l * 2 + kc) * WIDTH:(l * 2 + kc + 1) * WIDTH] = \
                w_hid[l, kc * 128:(kc + 1) * 128, :]
    whd = np.concatenate([np.asarray(inputs["w_rgb"], np.float32),
                          np.asarray(inputs["w_den"], np.float32)], axis=1)  # [256,4]
    whd8 = np.zeros((128, 2, 16), E4M3)
    whd8[:, :, 0:4] = whd.reshape(2, 128, 4).transpose(1, 0, 2).astype(E4M3)
    whd32 = np.empty((128, 8), np.float32)
    whd32[:, 0:4] = whd[0:128]
    whd32[:, 4:8] = whd[128:256]

    b_in = np.asarray(inputs["b_in"], np.float32)
    b_hid = np.asarray(inputs["b_hid"], np.float32)
    ball = np.zeros((128, 16), np.float32)
    for l in range(8):
        b = b_in if l == 0 else b_hid[l - 1]
        ball[:, 2 * l] = b[0:128]
        ball[:, 2 * l + 1] = b[128:256]
    b_rgb = np.asarray(inputs["b_rgb"], np.float32)
    b_den = np.asarray(inputs["b_den"], np.float32)
    bca = np.zeros((128, 1), np.float32)
    bca[0:S] = 0.5 * b_rgb[0]
    bca[S:128] = 0.5 * b_rgb[1]
    bcb = np.full((S, 1), 0.5 * b_rgb[2], np.float32)
    bcd = np.full((S, 1), b_den[0], np.float32)
    iota = np.arange(S, dtype=np.float32).reshape(S, 1)
    tris = (np.arange(S)[:, None] < np.arange(S)[None, :]).astype(np.float32)
    onesb = np.zeros((128, 2), np.float32)
    onesb[:S, 0] = 1.0
    onesb[S:, 1] = 1.0
    big = np.full((1, RPC), 1e10, np.float32)

    common = dict(brep=brep, win_r=win_r, win_32=win_32, whid8=whid8, whd8=whd8,
                  whid32=whid32, whd32=whd32, ball=ball, bca=bca, bcb=bcb,
                  bcd=bcd, iota=iota, tris=tris, onesb=onesb, big=big)
    in_maps = []
    for c in range(N_CORES):
        sl = slice(c * RPC, (c + 1) * RPC)
        m = dict(common)
        m["jitter_t"] = np.ascontiguousarray(jt[:, sl])
        m["rp4"] = np.ascontiguousarray(rp4[:, sl])
        m["rd4"] = np.ascontiguousarray(rd4[:, sl])
        in_maps.append(m)
    return in_maps


def kernel(**inputs):
    global LAST_EXEC_NS
    from concourse.bass_utils import run_bass_kernel_spmd
    if "nc" not in _CACHE:
        _CACHE["nc"] = _build_nc()
    nc = _CACHE["nc"]
    in_maps = _prep(inputs)
    res = run_bass_kernel_spmd(nc, in_maps, core_ids=list(range(N_CORES)))
    LAST_EXEC_NS = getattr(res, "exec_time_ns", None)
    if LAST_EXEC_NS is None:
        # no NTFF profiling in this environment: report the calibrated
        # single-core timeline-simulator estimate (SPMD — all cores equal)
        if "sim_ns" not in _CACHE:
            try:
                from concourse.timeline_sim import TimelineSim
                _CACHE["sim_ns"] = int(TimelineSim(nc, trace=False).simulate())
            except Exception:
                _CACHE["sim_ns"] = None
        LAST_EXEC_NS = _CACHE["sim_ns"]
    out = np.empty((N_CORES * RPC, 4), np.float32)
    for c in range(N_CORES):
        out[c * RPC:(c + 1) * RPC] = res.results[c]["out"].T
    return out


# revision 11
# speedup vs baseline: 1.2091x; 1.0115x over previous
"""NeRF render kernel v2 for 8 TRN2 NeuronCores (pure data parallel over rays).

Key speedups over v1 (810us -> ~452us timeline-sim):
- Hidden layers + heads in fp8-e4m3 (natural scale, no quant scaling needed);
  hidden layers use DoubleRow matmuls (full K=256 contraction in one MM).
  Sample 63 (1e10-delta, ReLU-sign-critical) stays full fp32: sample 62 runs
  as a solo fp8 stream and 63's fp32 layers are spread one-per-group across
  the schedule to avoid a serialization burst.
- Turns-domain Fourier encoding: the enc matmul computes u = pos*2^k/(2pi)
  (+0.25 for cos via a constant-1 moving row); range reduction is one ACT
  Copy (+MAGIC round) plus one DVE scalar_tensor_tensor giving round(u)-u,
  and Sin(scale=-2pi) restores the sign: sin(2pi*u) exactly (1-periodic).
- Encodings packed 2 samples per [124,512] tile; enc/L0 matmuls packed on
  row strips 0/32/64/96 (concurrent on HW via per-subarray row groups).
- ReLU+bias+fp8-quantize is one op per layer half, alternating DVE/ACT.
- Ray-position math, bulk weight-load DMA triggers, and SBUF-only composite
  ops run on the otherwise-idle GPSIMD engine; depth deltas precomputed.
- Head outputs batched 4 samples per PSUM bank (col strips), one copy + 4
  strided scatter DMAs per group. Exp/Tanh table set pre-warmed.
- Pair MLPs are generator-emitted one layer at a time and driven through a
  sliding 2-deep round-robin window that crosses group boundaries, so each
  engine always has another pair's ReLUs available while matmuls run and the
  pipeline never drains between groups: DVE/ACT occupancy ~93-94%.
"""
import os
import numpy as np
import ml_dtypes

NB = 10
ENC = 60
WIDTH = 256
S = 64
RPC = 512
N_CORES = 8
NEAR, FAR = 0.1, 4.0
TWO_PI = float(2 * np.pi)
MAGIC = float(1.5 * 2 ** 23)

LAST_EXEC_NS = None
_CACHE = {}


def _build_nc():
    import concourse.bacc as bacc
    import concourse.tile as tile
    from concourse import mybir

    dt = mybir.dt
    AF = mybir.ActivationFunctionType
    ALU = mybir.AluOpType
    f32 = dt.float32
    f32r = dt.float32r
    fp8 = dt.float8e4
    DR = mybir.MatmulPerfMode.DoubleRow

    nc = bacc.Bacc("TRN2", target_bir_lowering=False, debug=False,
                   num_devices=N_CORES)

    def din(name, shape, dtype=f32):
        return nc.dram_tensor(name, shape, dtype, kind="ExternalInput")

    d_jit = din("jitter_t", [S, RPC])
    d_rp4 = din("rp4", [128, RPC])      # 4x replicated rows: rp0,rp1,rp2,ones
    d_rd4 = din("rd4", [128, RPC])      # 4x replicated rows: rd0,rd1,rd2,zeros
    d_brep = din("brep", [128, ENC])    # 2^k/2pi pattern + 0.25 cos row
    d_win_r = din("win_r", [128, WIDTH], f32r)  # negated perm'd w_in, rows 0-59 & 64-123
    d_win_32 = din("win_32", [128, WIDTH])      # same data, fp32 for the 62/63 pair
    d_whid8 = din("whid8", [128, 2, 14 * 128], fp8)
    d_whd8 = din("whd8", [128, 2, 16], fp8)
    d_whid32 = din("whid32", [128, 7 * 2 * WIDTH])
    d_whd32 = din("whd32", [128, 8])
    d_ball = din("ball", [128, 16])
    d_bca = din("bca", [128, 1])
    d_bcb = din("bcb", [S, 1])
    d_bcd = din("bcd", [S, 1])
    d_iota = din("iota", [S, 1])
    d_tris = din("tris", [S, S])
    d_onesb = din("onesb", [128, 2])
    d_big = din("big", [1, RPC])
    d_out = nc.dram_tensor("out", [4, RPC], f32, kind="ExternalOutput")

    DEBUG = bool(os.environ.get("KERNEL_DEBUG"))
    dbg = {}
    if DEBUG:
        for nm, shp, dt_ in (("dbg_jpos", [128, RPC], f32), ("dbg_ue", [128, RPC], f32),
                             ("dbg_fr", [124, RPC], f32), ("dbg_enc", [124, RPC], f32),
                             ("dbg_x0", [128, 2, RPC], fp8), ("dbg_x7", [128, 2, RPC], fp8),
                             ("dbg_stg", [100, RPC], f32), ("dbg_den", [S, RPC], f32),
                             ("dbg_tau", [S, RPC], f32), ("dbg_wt", [S, RPC], f32)):
            dbg[nm] = nc.dram_tensor(nm, shp, dt_, kind="ExternalOutput")

    with tile.TileContext(nc) as tc:
        with (
            tc.tile_pool(name="static", bufs=1) as sp,
            tc.tile_pool(name="act", bufs=6) as ap,
            tc.tile_pool(name="jpos", bufs=16) as jp,
            tc.tile_pool(name="comp", bufs=1) as cp,
            tc.tile_pool(name="ps_m", bufs=1, space="PSUM") as pm_,
            tc.tile_pool(name="ps_h", bufs=1, space="PSUM") as ph_,
            tc.tile_pool(name="ps_l", bufs=6, space="PSUM") as pl,
        ):
            def load(dram, shape, dtype, tag):
                t = sp.tile(shape, dtype, tag=tag)
                nc.sync.dma_start(t[:], dram[:])
                return t

            # startup-critical loads first (gate the first encodings / L0)
            jt = load(d_jit, [S, RPC], f32, "jt")
            iota = load(d_iota, [S, 1], f32, "iota")
            brep = load(d_brep, [128, ENC], f32, "brep")
            win_r = load(d_win_r, [128, WIDTH], f32r, "win_r")
            win_32 = load(d_win_32, [128, WIDTH], f32, "win_32")
            ball = load(d_ball, [128, 16], f32, "ball")
            rp128 = load(d_rp4, [128, RPC], f32, "rp128")
            rd128 = load(d_rd4, [128, RPC], f32, "rd128")

            # depths = 0.1 + (3.9 * (idx + jitter)) / 64  (exact fp32 op order)
            # on DVE: it is idle at startup and SBUF-only ts ops run at 2x
            ddtmp = sp.tile([S, RPC], f32, tag="ddtmp")
            nc.vector.tensor_scalar(ddtmp[:], jt[:], iota[:], 3.9, ALU.add, ALU.mult)
            dd = sp.tile([S, RPC], f32, tag="dd")
            nc.vector.tensor_scalar(dd[:], ddtmp[:], float(1.0 / 64), 0.1, ALU.mult, ALU.add)

            def load_g(dram, shape, dtype, tag):
                t = sp.tile(shape, dtype, tag=tag)
                nc.gpsimd.dma_start(t[:], dram[:])
                return t

            whid8 = load_g(d_whid8, [128, 2, 14 * 128], fp8, "whid8")
            whd8 = load_g(d_whd8, [128, 2, 16], fp8, "whd8")
            whid32 = load_g(d_whid32, [128, 7 * 2 * WIDTH], f32, "whid32")
            whd32 = load_g(d_whd32, [128, 8], f32, "whd32")
            bca = load_g(d_bca, [128, 1], f32, "bca")
            bcb = load_g(d_bcb, [S, 1], f32, "bcb")
            bcd = load_g(d_bcd, [S, 1], f32, "bcd")
            tris = load_g(d_tris, [S, S], f32, "tris")
            onesb = load_g(d_onesb, [128, 2], f32, "onesb")

            # composite accumulators
            rgba = cp.tile([128, RPC], f32, tag="rgba")   # rows 0-63 rgb0, 64-127 rgb1
            rgbb = cp.tile([128, RPC], f32, tag="rgbb")   # rows 0-63 rgb2, 64-127 depths
            den = cp.tile([S, RPC], f32, tag="den")

            eng_ctr = [0]

            def relu_half(dst, src, bias_col, use_dve=None):
                """dst = fp8/f32(relu(src + bias)); alternates DVE/ACT."""
                if use_dve is None:
                    use_dve = bool(eng_ctr[0] & 1)
                    eng_ctr[0] += 1
                if use_dve:
                    nc.vector.tensor_scalar(dst, src, bias_col, 0.0, ALU.add, ALU.max)
                else:
                    nc.scalar.activation(dst, src, AF.Relu, bias=bias_col)

            def emit_pair(sA, sB, enc, g, layer_sync=None):
                """MLP for a 2-sample fp8 pair (f32r L0, DoubleRow hidden).
                If layer_sync is a generator-style callback it is invoked after
                each layer to interleave another pair's layer emission."""
                xA = ap.tile([128, 2, RPC], fp8, tag="x8a")
                xB = ap.tile([128, 2, RPC], fp8, tag="x8b")
                for l in range(8):
                    pms = []
                    for mc in range(2):
                        pA = pl.tile([128, RPC], f32, tag="lp", name="pA")
                        pB = pl.tile([128, RPC], f32, tag="lp", name="pB")
                        if l == 0:
                            nc.tensor.matmul(pA[:], win_r[0:ENC, mc * 128:(mc + 1) * 128],
                                             enc[0:ENC, :], start=True, stop=True,
                                             tile_position=(0, 0))
                            nc.tensor.matmul(pB[:], win_r[64:64 + ENC, mc * 128:(mc + 1) * 128],
                                             enc[64:64 + ENC, :], start=True, stop=True,
                                             tile_position=(64, 0))
                        else:
                            ch = ((l - 1) * 2 + mc) * 128
                            nc.tensor.matmul(pA[:], whid8[:, :, ch:ch + 128],
                                             xA[:, :, :], start=True, stop=True,
                                             perf_mode=DR)
                            nc.tensor.matmul(pB[:], whid8[:, :, ch:ch + 128],
                                             xB[:, :, :], start=True, stop=True,
                                             perf_mode=DR)
                        pms.append((pA, pB))
                    nxA = ap.tile([128, 2, RPC], fp8, tag="x8a")
                    nxB = ap.tile([128, 2, RPC], fp8, tag="x8b")
                    for mc in range(2):
                        pA, pB = pms[mc]
                        col = ball[:, 2 * l + mc:2 * l + mc + 1]
                        relu_half(nxA[:, mc, :], pA[:], col)
                        relu_half(nxB[:, mc, :], pB[:], col)
                    xA, xB = nxA, nxB
                    if DEBUG and sA == 0 and l == 0:
                        nc.sync.dma_start(dbg["dbg_x0"][:], xA[:])
                    if DEBUG and sA == 0 and l == 7:
                        nc.sync.dma_start(dbg["dbg_x7"][:], xA[:])
                    if layer_sync is not None:
                        yield
                hpg = _head_ps[g]
                qA, qB = 32 * (sA % 4), 32 * (sB % 4)
                for kc in range(2):
                    nc.tensor.matmul(hpg[qA:qA + 4, :], whd8[:, kc, 0:4], xA[:, kc, :],
                                     start=(kc == 0), stop=(kc == 1),
                                     tile_position=(0, qA))
                for kc in range(2):
                    nc.tensor.matmul(hpg[qB:qB + 4, :], whd8[:, kc, 0:4], xB[:, kc, :],
                                     start=(kc == 0), stop=(kc == 1),
                                     tile_position=(0, qB))
                if layer_sync is not None:
                    yield

            def run_pair(sA, sB, enc, g):
                """Non-interleaved execution of emit_pair (drain generator)."""
                for _ in emit_pair(sA, sB, enc, g, layer_sync=True):
                    pass

            def run_two_pairs(p1, p2, g):
                """Interleave two pairs layer-by-layer for cross-pair overlap."""
                g1 = emit_pair(*p1, g, layer_sync=True)
                g2 = emit_pair(*p2, g, layer_sync=True)
                done1 = done2 = False
                while not (done1 and done2):
                    if not done1:
                        done1 = next(g1, "end") == "end"
                    if not done2:
                        done2 = next(g2, "end") == "end"

            def emit_62_solo(enc63, g):
                """Sample 62 alone: fp8 MLP off the shared f32 enc63 tile."""
                xA = ap.tile([128, 2, RPC], fp8, tag="x8a")
                for l in range(8):
                    pms = []
                    for mc in range(2):
                        pA = pl.tile([128, RPC], f32, tag="lp", name="pA")
                        if l == 0:
                            nc.tensor.matmul(pA[:], win_32[0:ENC, mc * 128:(mc + 1) * 128],
                                             enc63[0:ENC, :], start=True, stop=True,
                                             tile_position=(0, 0))
                        else:
                            ch = ((l - 1) * 2 + mc) * 128
                            nc.tensor.matmul(pA[:], whid8[:, :, ch:ch + 128],
                                             xA[:, :, :], start=True, stop=True,
                                             perf_mode=DR)
                        pms.append(pA)
                    nxA = ap.tile([128, 2, RPC], fp8, tag="x8a")
                    for mc in range(2):
                        col = ball[:, 2 * l + mc:2 * l + mc + 1]
                        relu_half(nxA[:, mc, :], pms[mc][:], col, mc == 0)
                    xA = nxA
                hpg = _head_ps[g]
                for kc in range(2):
                    nc.tensor.matmul(hpg[64:68, :], whd8[:, kc, 0:4], xA[:, kc, :],
                                     start=(kc == 0), stop=(kc == 1),
                                     tile_position=(0, 64))

            x63 = [None]

            def emit_63_layer(l, enc63):
                """One fp32 layer of sample 63, interleaved into the schedule."""
                nxt = sp.tile([128, 2, RPC], f32, tag=f"x63_{l % 2}", name="x63")
                pms = []
                for mc in range(2):
                    pB = pl.tile([128, RPC], f32, tag="lp", name="p63")
                    if l == 0:
                        nc.tensor.matmul(pB[:], win_32[64:64 + ENC, mc * 128:(mc + 1) * 128],
                                         enc63[64:64 + ENC, :], start=True, stop=True,
                                         tile_position=(64, 0))
                    else:
                        for kc in range(2):
                            col = ((l - 1) * 2 + kc) * WIDTH + mc * 128
                            nc.tensor.matmul(pB[:], whid32[:, col:col + 128],
                                             x63[0][:, kc, :], start=(kc == 0), stop=(kc == 1))
                    pms.append(pB)
                for mc in range(2):
                    col = ball[:, 2 * l + mc:2 * l + mc + 1]
                    relu_half(nxt[:, mc, :], pms[mc][:], col, mc == 0)
                x63[0] = nxt

            def emit_63_heads():
                hp63 = pl.tile([128, RPC], f32, tag="lp", name="hp63")
                for kc in range(2):
                    nc.tensor.matmul(hp63[0:4, :], whd32[:, kc * 4:kc * 4 + 4],
                                     x63[0][:, kc, :], start=(kc == 0), stop=(kc == 1),
                                     tile_position=(0, 0))
                s63 = cp.tile([4, RPC], f32, tag="stg63")
                nc.scalar.copy(s63[:], hp63[0:4, :])
                nc.sync.dma_start(rgba[63:64, :], s63[0:1, :])
                nc.sync.dma_start(rgba[127:128, :], s63[1:2, :])
                nc.sync.dma_start(rgbb[63:64, :], s63[2:3, :])
                nc.sync.dma_start(den[63:64, :], s63[3:4, :])

            _head_ps = {}
            gorder = [15] + list(range(15))
            jposs = {}
            for idx, g in enumerate(gorder):
                s0 = 4 * g
                dd4 = ap.tile([128, RPC], f32, tag="dd4")
                # i=3 rows multiply rd128 rows that are 0; any finite fill works
                # (they must be written: x*0 of uninitialized NaN poisons jpos)
                for i in range(4):
                    nc.sync.dma_start(dd4[i::32, :], dd[s0:s0 + 4, :])
                eng = nc.vector if idx < 2 else nc.gpsimd
                jtmp = ap.tile([128, RPC], f32, tag="jtmp")
                eng.tensor_tensor(jtmp[:], dd4[:], rd128[:], ALU.mult)
                jpos = jp.tile([128, RPC], f32, tag="jpos", name="jpos")
                eng.tensor_tensor(jpos[:], jtmp[:], rp128[:], ALU.add)
                if DEBUG and g == 0:
                    nc.sync.dma_start(dbg["dbg_jpos"][:], jpos[:])
                jposs[g] = jpos

            enc63 = sp.tile([124, RPC], f32, tag="enc63")

            # deltas depend only on depths; emitted here so the DMAs/sub run
            # during the MLP, off the startup-critical SP/DVE queues
            ddsh = sp.tile([S, RPC], f32, tag="ddsh")
            nc.sync.dma_start(ddsh[0:63, :], dd[1:64, :])
            nc.sync.dma_start(ddsh[63:64, :], d_big[:])
            delt = sp.tile([S, RPC], f32, tag="delt")
            nc.gpsimd.tensor_tensor(delt[:], ddsh[:], dd[:], ALU.subtract)

            active = []
            group_left = {}

            def finish_group(g):
                s0 = 4 * g
                stg = ap.tile([100, RPC], f32, tag="stg", name="stg")
                nc.scalar.copy(stg[:], _head_ps[g][0:100, :])
                if DEBUG and g == 0:
                    nc.sync.dma_start(dbg["dbg_stg"][:], stg[:])
                nc.sync.dma_start(rgba[s0:s0 + 4, :], stg[0::32, :])
                nc.sync.dma_start(rgba[S + s0:S + s0 + 4, :], stg[1::32, :])
                nc.sync.dma_start(rgbb[s0:s0 + 4, :], stg[2::32, :])
                nc.sync.dma_start(den[s0:s0 + 4, :], stg[3::32, :])

            def step_all_once():
                for item in list(active):
                    gen, gg = item
                    if next(gen, "end") == "end":
                        active.remove(item)
                        group_left[gg] -= 1
                        if group_left[gg] == 0:
                            finish_group(gg)

            for gi, g in enumerate(gorder):
                if 1 <= gi <= 8:
                    emit_63_layer(gi - 1, enc63)
                elif gi == 9:
                    emit_63_heads()
                s0 = 4 * g
                jpos = jposs[g]
                _head_ps[g] = ph_.tile([128, RPC], f32, tag="hp", name="hpg")

                ues = []
                for pr in range(2):
                    jA, jB = 2 * pr, 2 * pr + 1
                    ue = pm_.tile([128, RPC], f32, tag="m", name="ue")
                    nc.tensor.matmul(ue[0:ENC, :], brep[32 * jA:32 * jA + 4, :],
                                     jpos[32 * jA:32 * jA + 4, :], start=True,
                                     stop=True, tile_position=(32 * jA, 0))
                    nc.tensor.matmul(ue[64:64 + ENC, :], brep[32 * jB:32 * jB + 4, :],
                                     jpos[32 * jB:32 * jB + 4, :], start=True,
                                     stop=True, tile_position=(32 * jB, 64))
                    ues.append(ue)
                prs = []
                solo = False
                for pr in range(2):
                    sA, sB = s0 + 2 * pr, s0 + 2 * pr + 1
                    # rnm = fl(u + MAGIC) on ACT (Copy is exact); then on DVE
                    # fr_neg = (rnm - MAGIC) - u = round(u) - u; Sin scale -2pi
                    # flips the sign back: sin(2pi*(u - round(u))) = sin(2pi*u).
                    rnm = ap.tile([124, RPC], f32, tag="rnm")
                    nc.scalar.activation(rnm[:], ues[pr][0:124, :], AF.Copy, bias=MAGIC)
                    fr = ap.tile([124, RPC], f32, tag="fr")
                    nc.vector.scalar_tensor_tensor(fr[:], rnm[:], -MAGIC, ues[pr][0:124, :],
                                                   ALU.add, ALU.subtract)
                    enc = enc63 if sB == 63 else ap.tile([124, RPC], f32r, tag="enc")
                    nc.scalar.activation(enc[:], fr[:], AF.Sin, scale=-TWO_PI)
                    if DEBUG and sA == 0:
                        uec = ap.tile([128, RPC], f32, tag="uec")
                        nc.vector.tensor_copy(uec[:], ues[pr][:])
                        nc.sync.dma_start(dbg["dbg_ue"][:], uec[:])
                        nc.sync.dma_start(dbg["dbg_fr"][:], fr[:])
                        nc.sync.dma_start(dbg["dbg_enc"][:], enc[:].bitcast(f32))
                    if sB == 63:
                        solo = True
                    else:
                        prs.append((sA, sB, enc))
                group_left[g] = len(prs)
                for p in prs:
                    while len(active) >= 2:
                        step_all_once()
                    active.append((emit_pair(*p, g, layer_sync=True), g))
                if solo:
                    emit_62_solo(enc63, g)
            while active:
                step_all_once()

            # warm the exp/tanh activation table set off the critical tail
            warm = cp.tile([1, 8], f32, tag="warm")
            nc.scalar.activation(warm[:], dd[0:1, 0:8], AF.Exp)
            nc.scalar.activation(warm[:], dd[0:1, 0:8], AF.Tanh)

            # ---- head activations ----
            # rgb = 0.5 + 0.5*tanh(0.5*z + 0.5*b_rgb); den = relu(z + b_den)
            tmpa = cp.tile([128, RPC], f32, tag="tmpa")
            nc.scalar.activation(tmpa[:], rgba[:], AF.Tanh, bias=bca[:], scale=0.5)
            nc.gpsimd.tensor_scalar(rgba[:], tmpa[:], 0.5, 0.5, ALU.mult, ALU.add)
            tmpb = cp.tile([S, RPC], f32, tag="tmpb")
            nc.scalar.activation(tmpb[:], rgbb[0:S, :], AF.Tanh, bias=bcb[:], scale=0.5)
            nc.gpsimd.tensor_scalar(rgbb[0:S, :], tmpb[:], 0.5, 0.5, ALU.mult, ALU.add)
            denr = cp.tile([S, RPC], f32, tag="denr")
            nc.vector.tensor_scalar(denr[:], den[:], bcd[:], 0.0, ALU.add, ALU.max)

            # ---- volume rendering composite ----
            tau = cp.tile([S, RPC], f32, tag="tau")
            nc.vector.tensor_tensor(tau[:], denr[:], delt[:], ALU.mult)
            exclp = pl.tile([128, RPC], f32, tag="lp", name="exclp")
            nc.tensor.matmul(exclp[0:S, :], tris[:], tau[:], start=True, stop=True)
            inc = cp.tile([S, RPC], f32, tag="inc")
            nc.vector.tensor_tensor(inc[:], exclp[0:S, :], tau[:], ALU.add)
            exc2 = cp.tile([S, RPC], f32, tag="exc2")
            nc.vector.tensor_tensor(exc2[:], inc[:], tau[:], ALU.subtract)
            trans = cp.tile([S, RPC], f32, tag="trans")
            nc.scalar.activation(trans[:], exc2[:], AF.Exp, scale=-1.0)
            ee = cp.tile([S, RPC], f32, tag="ee")
            nc.scalar.activation(ee[:], tau[:], AF.Exp, scale=-1.0)
            alpha = cp.tile([S, RPC], f32, tag="alpha")
            nc.gpsimd.tensor_scalar(alpha[:], ee[:], -1.0, 1.0, ALU.mult, ALU.add)
            wt = cp.tile([S, RPC], f32, tag="wt")
            nc.vector.tensor_tensor(wt[:], alpha[:], trans[:], ALU.mult)
            if DEBUG:
                nc.sync.dma_start(dbg["dbg_den"][:], den[:])
                nc.sync.dma_start(dbg["dbg_tau"][:], tau[:])
                nc.sync.dma_start(dbg["dbg_wt"][:], wt[:])
            w2 = cp.tile([128, RPC], f32, tag="w2")
            nc.sync.dma_start(w2[0:S, :], wt[:])
            nc.sync.dma_start(w2[S:128, :], wt[:])
            nc.sync.dma_start(rgbb[S:128, :], dd[:])
            wa = cp.tile([128, RPC], f32, tag="wa")
            nc.vector.tensor_tensor(wa[:], w2[:], rgba[:], ALU.mult)
            wb = cp.tile([128, RPC], f32, tag="wb")
            nc.gpsimd.tensor_tensor(wb[:], w2[:], rgbb[:], ALU.mult)
            redp = pl.tile([128, RPC], f32, tag="lp", name="redp")
            nc.tensor.matmul(redp[0:2, :], onesb[:], wa[:], start=True, stop=True)
            nc.tensor.matmul(redp[32:34, :], onesb[:], wb[:], start=True, stop=True)
            outsb = cp.tile([S, RPC], f32, tag="outsb")
            nc.vector.tensor_copy(outsb[0:2, :], redp[0:2, :])
            nc.vector.tensor_copy(outsb[32:34, :], redp[32:34, :])
            nc.sync.dma_start(d_out[0:2, :], outsb[0:2, :])
            nc.sync.dma_start(d_out[2:4, :], outsb[32:34, :])

    nc.compile()
    return nc


def _prep(inputs):
    E4M3 = ml_dtypes.float8_e4m3fn
    jt = np.ascontiguousarray(np.asarray(inputs["jitter"], np.float32).T)
    rpt = np.asarray(inputs["ray_pos"], np.float32).T
    rdt = np.asarray(inputs["ray_dir"], np.float32).T
    rp4 = np.empty((128, 4096), np.float32)
    rd4 = np.zeros((128, 4096), np.float32)
    for j in range(4):
        rp4[32 * j:32 * j + 3] = rpt
        rp4[32 * j + 3] = 1.0
        rd4[32 * j:32 * j + 3] = rdt
    rp4[[r for r in range(128) if r % 32 > 3]] = 0.0

    w_in = np.asarray(inputs["w_in"], np.float32)
    perm = np.empty(ENC, np.int64)
    for r in range(ENC):
        base = 0 if r < 30 else 10
        rr = r % 30
        perm[r] = (rr // 10) * 20 + base + (rr % 10)
    win_p = w_in[perm]  # frac is round-to-nearest: sin(2*pi*(u-round(u))) = sin(2*pi*u)
    win_ext = np.zeros((128, WIDTH), np.float32)
    win_ext[0:ENC] = win_p
    win_ext[64:64 + ENC] = win_p
    win_r = win_ext
    win_32 = win_ext

    # brep: u = pos_i * (2^k/2pi)  (+0.25 for cos cols via the ones row)
    ck = (2.0 ** np.arange(NB)) / (2 * np.pi)
    brep = np.zeros((128, ENC), np.float32)
    for r in range(ENC):
        rr = r % 30
        i, k = rr // 10, rr % 10
        for j in range(4):
            brep[32 * j + i, r] = np.float32(ck[k])
            if r >= 30:
                brep[32 * j + 3, r] = 0.25

    w_hid = np.asarray(inputs["w_hid"], np.float32)
    # fp8 layout [128(p), 2(i=k-half), 14(l*2+m)*128(c)]
    whid8 = np.ascontiguousarray(
        w_hid.reshape(7, 2, 128, 2, 128).transpose(2, 1, 0, 3, 4)
        .reshape(128, 2, 7 * 2 * 128)).astype(E4M3)
    whid32 = np.empty((128, 7 * 2 * WIDTH), np.float32)
    for l in range(7):
        for kc in range(2):
            whid32[:, (l * 2 + kc) * WIDTH:(l * 2 + kc + 1) * WIDTH] = \
                w_hid[l, kc * 128:(kc + 1) * 128, :]
    whd = np.concatenate([np.asarray(inputs["w_rgb"], np.float32),
                          np.asarray(inputs["w_den"], np.float32)], axis=1)  # [256,4]
    whd8 = np.zeros((128, 2, 16), E4M3)
    whd8[:, :, 0:4] = whd.reshape(2, 128, 4).transpose(1, 0, 2).astype(E4M3)
    whd32 = np.empty((128, 8), np.float32)
    whd32[:, 0:4] = whd[0:128]
    whd32[:, 4:8] = whd[128:256]

    b_in = np.asarray(inputs["b_in"], np.float32)
    b_hid = np.asarray(inputs["b_hid"], np.float32)
    ball = np.zeros((128, 16), np.float32)
    for l in range(8):
        b = b_in if l == 0 else b_hid[l - 1]
        ball[:, 2 * l] = b[0:128]
        ball[:, 2 * l + 1] = b[128:256]
    b_rgb = np.asarray(inputs["b_rgb"], np.float32)
    b_den = np.asarray(inputs["b_den"], np.float32)
    bca = np.zeros((128, 1), np.float32)
    bca[0:S] = 0.5 * b_rgb[0]
    bca[S:128] = 0.5 * b_rgb[1]
    bcb = np.full((S, 1), 0.5 * b_rgb[2], np.float32)
    bcd = np.full((S, 1), b_den[0], np.float32)
    iota = np.arange(S, dtype=np.float32).reshape(S, 1)
    tris = (np.arange(S)[:, None] < np.arange(S)[None, :]).astype(np.float32)
    onesb = np.zeros((128, 2), np.float32)
    onesb[:S, 0] = 1.0
    onesb[S:, 1] = 1.0
    big = np.full((1, RPC), 1e10, np.float32)

    common = dict(brep=brep, win_r=win_r, win_32=win_32, whid8=whid8, whd8=whd8,
                  whid32=whid32, whd32=whd32, ball=ball, bca=bca, bcb=bcb,
                  bcd=bcd, iota=iota, tris=tris, onesb=onesb, big=big)
    in_maps = []
    for c in range(N_CORES):
        sl = slice(c * RPC, (c + 1) * RPC)
        m = dict(common)
        m["jitter_t"] = np.ascontiguousarray(jt[:, sl])
        m["rp4"] = np.ascontiguousarray(rp4[:, sl])
        m["rd4"] = np.ascontiguousarray(rd4[:, sl])
        in_maps.append(m)
    return in_maps


def kernel(**inputs):
    global LAST_EXEC_NS
    from concourse.bass_utils import run_bass_kernel_spmd
    if "nc" not in _CACHE:
        _CACHE["nc"] = _build_nc()
    nc = _CACHE["nc"]
    in_maps = _prep(inputs)
    res = run_bass_kernel_spmd(nc, in_maps, core_ids=list(range(N_CORES)))
    LAST_EXEC_NS = getattr(res, "exec_time_ns", None)
    if LAST_EXEC_NS is None:
        # no NTFF profiling in this environment: report the calibrated
        # single-core timeline-simulator estimate (SPMD — all cores equal)
        if "sim_ns" not in _CACHE:
            try:
                from concourse.timeline_sim import TimelineSim
                _CACHE["sim_ns"] = int(TimelineSim(nc, trace=False).simulate())
            except Exception:
                _CACHE["sim_ns"] = None
        LAST_EXEC_NS = _CACHE["sim_ns"]
    out = np.empty((N_CORES * RPC, 4), np.float32)
    for c in range(N_CORES):
        out[c * RPC:(c + 1) * RPC] = res.results[c]["out"].T
    return out
